# revision 44
# baseline (speedup 1.0000x reference)
"""Trainium2 Bass kernel for nn_EncoderLayer (dense transformer encoder layer
with static-expansion attention-like block + FF), data-parallel over 8 cores.

Contract: kernel(**inputs) takes FULL unsharded inputs (as in setup_inputs()),
returns the FULL (64, 256, 512) float32 output.

v2: bf16 matmuls (FWL weight loads), Wk folded into q_tab on host, DMA-engine
transposes (XBAR) instead of PE transposes, biases via broadcast tiles + DVE,
bw denominators via DVE reduce, batch elements processed in pairs so the
zfull/FF matmul streams run at N=512, single merged loop (no y2 roundtrip),
software-pipelined so the FF block of pair p-1 fills the gather latency of
pair p.
"""

import sys

for _p in ("/opt/trn_rl_repo",):
    if _p not in sys.path:
        sys.path.insert(0, _p)

import numpy as np
from ml_dtypes import bfloat16 as np_bf16

import concourse.bass as bass
import concourse.mybir as mybir
import concourse.tile as tile
from concourse.vector_clock import ScopedClock

F32 = mybir.dt.float32
BF16 = mybir.dt.bfloat16
I32 = mybir.dt.int32
OP = mybir.AluOpType
AF = mybir.ActivationFunctionType
AX = mybir.AxisListType

D = 512          # d_model
DFF = 2048       # d_ff
N = 992          # n experts
NPAD = 1024
L = 256          # enc len
BS = 64
NCORES = 8
BPC = BS // NCORES  # batch elements per core
EPS = 1e-9
LN_EPS = 1e-5

KD = D // 128     # 4 k-chunks over d_model
LT = L // 128     # 2 l-chunks
NMO = 8           # n-chunks over N (7x128 + 96)
NSZ = [128] * 7 + [96]
NOFF = [128 * i for i in range(8)]
KF = DFF // 128   # 16 chunks over d_ff

# bias6 row indices
B_A, B_GA, B_B, B_GB, B_S, B_F2 = range(6)
# w5 slot indices
W_A, W_GA, W_B, W_GB, W_S = range(5)


class SplitDrainTC(tile.TileContext):
    """TileContext whose exit drain splits semaphore waits across nop
    instructions (this walrus build rejects >2 sync waits on one Drain)."""

    def _drain_and_barrier(self, tick_clock, wait_clock):
        nc = self.nc
        probe = nc.sync.nop(nofuse=True)
        wait_clock.add_sem_waits(probe.ins, ScopedClock({None: tick_clock.global_clock}))
        si = probe.ins.sync_info
        waits = list(si.on_wait) if si and si.on_wait else []
        if len(waits) > 1:
            si.on_wait = waits[:1]
            sems_by_name = {h.name: h for h in self.sems.allocated().values()}
            for w in waits[1:]:
                n2 = nc.sync.nop(nofuse=True)
                n2.wait_op(sems_by_name[w.ant_name], w.wait_value, "sem-ge")
        nc.sync.drain()
        nc.all_engine_barrier()
        popped = nc._tile_sem_poison_stack.pop()
        assert popped is self._sem_poison
        nc.clear_and_free_semaphores(list(self.sems.allocated().values()))
        nc.all_engine_barrier()


def _split_excess_waits(nc, cap=2):
    """This walrus build rejects instructions carrying more than ~2 sync
    waits. Hoist excess waits onto same-engine nop instructions inserted
    immediately before the offending instruction (engine program order is
    bb order, so the nop's waits complete first)."""
    import bass_rust
    for f in nc.m.functions:
        for bb in f.blocks:
            over = [inst for inst in bb.instructions
                    if inst.sync_info and inst.sync_info.on_wait
                    and len(inst.sync_info.on_wait) > cap]
            if not over:
                continue
            carriers = {}
            for inst in over:
                waits = list(inst.sync_info.on_wait)
                inst.sync_info.on_wait = waits[:cap]
                rest = waits[cap:]
                lst = []
                for i in range(0, len(rest), cap):
                    nop = nc.engines[inst.engine].nop(nofuse=True)
                    cur = nc.cur_bb.bb
                    assert cur.instructions[-1] is nop.ins
                    cur.instructions.pop()
                    nop.ins.sync_info = bass_rust.SyncInfo(
                        on_wait=rest[i:i + cap], on_update=[])
                    lst.append(nop.ins)
                carriers[inst.name] = lst
            out = []
            for inst in bb.instructions:
                out.extend(carriers.get(inst.name, ()))
                out.append(inst)
            bb.instructions[:] = out


def build_program(n_elems=BPC):
    """Single-core SPMD program; see kernel() for the per-core input map."""
    nc = bass.Bass("TRN2", target_bir_lowering=False, debug=False)

    x_d = nc.dram_tensor("x", [n_elems, L, D], F32, kind="ExternalInput").ap()
    nidx_d = nc.dram_tensor("nidx", [n_elems, NPAD], I32, kind="ExternalInput").ap()
    mask_d = nc.dram_tensor("mask", [n_elems, NPAD, L], BF16, kind="ExternalInput").ap()
    qWT_d = nc.dram_tensor("qWT", [D, N], BF16, kind="ExternalInput").ap()
    w5_d = nc.dram_tensor("w5", [5, D, D], BF16, kind="ExternalInput").ap()
    btab_d = nc.dram_tensor("b_tab", [N, D], BF16, kind="ExternalInput").ap()
    bias6_d = nc.dram_tensor("bias6", [6, D], BF16, kind="ExternalInput").ap()
    zbcol_d = nc.dram_tensor("zbcol", [128, NMO], F32, kind="ExternalInput").ap()
    bf1col_d = nc.dram_tensor("bf1col", [128, KF], F32, kind="ExternalInput").ap()
    wf1_d = nc.dram_tensor("wf1", [D, DFF], BF16, kind="ExternalInput").ap()
    wf2_d = nc.dram_tensor("wf2", [DFF, D], BF16, kind="ExternalInput").ap()
    ident_d = nc.dram_tensor("ident", [128, 128], BF16, kind="ExternalInput").ap()
    out_d = nc.dram_tensor("out", [n_elems, L, D], BF16, kind="ExternalOutput").ap()

    with SplitDrainTC(nc) as tc:
        _emit(nc, tc, n_elems, x_d, nidx_d, mask_d, qWT_d, w5_d, btab_d,
              bias6_d, zbcol_d, bf1col_d, wf1_d, wf2_d, ident_d, out_d)
    _split_excess_waits(nc, cap=1)
    return nc


def _layer_norm(nc, pool_small, xn, x_sb, eps_tile, uid):
    """xn[:, lt, :] = (x - mean)/sqrt(var + LN_EPS)."""
    for lt in range(LT):
        stats = pool_small.tile([128, 6], F32, tag="ln_stats")
        nc.vector.bn_stats(stats[:], x_sb[:, lt, :])
        aggr = pool_small.tile([128, 2], F32, tag="ln_aggr")
        nc.vector.bn_aggr(aggr[:], stats[:])
        sv = pool_small.tile([128, 1], F32, tag="ln_sv")
        nc.scalar.activation(sv[:], aggr[:, 1:2], AF.Sqrt, bias=eps_tile[:])
        rstd = pool_small.tile([128, 1], F32, tag="ln_rstd")
        nc.vector.reciprocal(rstd[:], sv[:])
        nmr = pool_small.tile([128, 1], F32, tag="ln_nmr")
        nc.vector.tensor_scalar(out=nmr[:], in0=aggr[:, 0:1], scalar1=rstd[:],
                                scalar2=-1.0, op0=OP.mult, op1=OP.mult)
        if lt == 0:
            nc.scalar.activation(xn[:, lt, :], x_sb[:, lt, :], AF.Identity,
                                 bias=nmr[:], scale=rstd[:])
        else:
            nc.vector.tensor_scalar(out=xn[:, lt, :], in0=x_sb[:, lt, :],
                                    scalar1=rstd[:], scalar2=nmr[:],
                                    op0=OP.mult, op1=OP.add)


class _Weights:
    pass


def _emit(nc, tc, n_elems, x_d, nidx_d, mask_d, qWT_d, w5_d, btab_d,
          bias6_d, zbcol_d, bf1col_d, wf1_d, wf2_d, ident_d, out_d):
    from contextlib import ExitStack

    top = ExitStack()
    with top:
        w = _Weights()
        wpool = top.enter_context(tc.tile_pool(name="w", bufs=1))
        w.eps = wpool.tile([128, 1], F32)
        nc.vector.memset(w.eps[:], LN_EPS)
        w.ident = wpool.tile([128, 128], BF16)
        nc.sync.dma_start(w.ident[:], ident_d)
        w.ones_row = wpool.tile([1, 128], BF16)
        nc.vector.memset(w.ones_row[:], 1.0)
        w.btab_d = btab_d

        def _load_weights():
            w.qWT = wpool.tile([128, KD, N], BF16, name="qWT_sb")
            nc.sync.dma_start(w.qWT[:], qWT_d.rearrange("(k p) n -> p k n", p=128))
            w.w5 = wpool.tile([128, 5, KD, D], BF16, name="w5_sb")
            for wi in range(5):
                nc.scalar.dma_start(w.w5[:, wi, :, :],
                                    w5_d[wi].rearrange("(k p) n -> p k n", p=128))
            w.wf1 = wpool.tile([128, KD, DFF], BF16, name="wf1_sb")
            nc.scalar.dma_start(w.wf1[:, :, :DFF // 2],
                                wf1_d[:, :DFF // 2].rearrange("(k p) n -> p k n", p=128))
            nc.scalar.dma_start(w.wf1[:, :, DFF // 2:],
                                wf1_d[:, DFF // 2:].rearrange("(k p) n -> p k n", p=128))
            w.wf2 = wpool.tile([128, KF, D], BF16, name="wf2_sb")
            nc.scalar.dma_start(w.wf2[:], wf2_d.rearrange("(k p) n -> p k n", p=128))
            w.bias6 = wpool.tile([1, 6, D], BF16, name="bias6_sb")
            nc.scalar.dma_start(w.bias6[:], bias6_d.rearrange("(o s) d -> o s d", o=1))
            w.zbcol = wpool.tile([128, NMO], F32, name="zbcol_sb")
            nc.scalar.dma_start(w.zbcol[:], zbcol_d)
            w.bf1col = wpool.tile([128, KF], F32, name="bf1col_sb")
            nc.scalar.dma_start(w.bf1col[:], bf1col_d)

        ps = top.enter_context(tc.tile_pool(name="ps", bufs=7, space="PSUM"))
        pstr = top.enter_context(tc.tile_pool(name="pstr", bufs=1, space="PSUM"))
        act = top.enter_context(tc.tile_pool(name="act", bufs=1))
        small = top.enter_context(tc.tile_pool(name="small", bufs=2))
        dram = top.enter_context(tc.tile_pool(name="dram", bufs=2, space="DRAM"))

        n_pairs = n_elems // 2
        prev = None
        nxt = _pre(nc, 0, act, small, pstr, w, x_d, nidx_d, mask_d)
        _load_weights()
        for p in range(n_pairs):
            st = nxt
            _zfull_part(nc, p, st, act, small, ps, dram, w)
            if prev is not None:
                _ff(nc, prev, act, small, ps, pstr, w, out_d)
            _sel_part(nc, p, st, act, small, ps, w)
            if p + 1 < n_pairs:
                nxt = _pre(nc, p + 1, act, small, pstr, w, x_d, nidx_d, mask_d)
            _mid(nc, p, st, act, small, ps, pstr, w, x_d)
            prev = st
        _ff(nc, prev, act, small, ps, pstr, w, out_d)


class _PairState:
    pass


def _pre(nc, p, act, small, pstr, w, x_d, nidx_d, mask_d):
    """Pair-p input loads + LN1 + x2T transposes (hoisted one stage early)."""
    st = _PairState()
    st.p = p
    b0, b1 = 2 * p, 2 * p + 1
    st.bs = (b0, b1)

    st.idx = []
    st.mask = []
    xs = []
    for e, b in enumerate(st.bs):
        xt = act.tile([128, LT, D], F32, tag=f"x{e}", bufs=1,
                      name=f"x_{p}_{e}")
        nc.sync.dma_start(xt[:], x_d[b].rearrange("(lt p) d -> p lt d", p=128))
        xs.append(xt)
        it = act.tile([128, NMO], I32, tag=f"idx{e}", bufs=2,
                      name=f"idx_{p}_{e}")
        nc.scalar.dma_start(it[:], nidx_d[b].rearrange("(a p) -> p a", p=128))
        st.idx.append(it)
        mt = act.tile([128, NMO, L], BF16, tag=f"mask{e}", bufs=1,
                      name=f"mask_{p}_{e}")
        nc.sync.dma_start(mt[:], mask_d[b].rearrange("(mo p) l -> p mo l", p=128))
        st.mask.append(mt)

    st.x2T = act.tile([128, KD, 2 * L], BF16, tag="x2T", bufs=1,
                      name=f"x2T_{p}")
    for e in range(2):
        xn = act.tile([128, LT, D], BF16, tag=f"xn{e}", bufs=1,
                      name=f"xn_{p}_{e}")
        _layer_norm(nc, small, xn, xs[e], w.eps, f"ln1_{p}_{e}")
        for lt in range(LT):
            ptr = pstr.tile([128, KD, 128], BF16, tag="tr",
                            name=f"trx_{p}_{e}_{lt}")
            for ko in range(KD):
                nc.tensor.transpose(ptr[:, ko, :],
                                    xn[:, lt, ko * 128:(ko + 1) * 128],
                                    w.ident[:])
            off = e * L + lt * 128
            if lt == 0:
                nc.scalar.copy(st.x2T[:, :, off:off + 128], ptr[:, :, :])
            else:
                nc.vector.tensor_copy(st.x2T[:, :, off:off + 128], ptr[:, :, :])
    return st


def _zfull_part(nc, p, st, act, small, ps, dram, w):
    """zfull matmuls + stores + z gathers."""
    # ---- zfull (pair): z[n, 2L] = qW @ xn^T + zb ----
    zf = [dram.tile([N, L], BF16, tag=f"zfull{e}", bufs=2, name=f"zf_{p}_{e}")
          for e in range(2)]
    st.z = [act.tile([128, NMO, L], BF16, tag=f"z{e}", bufs=1,
                     name=f"z_{p}_{e}") for e in range(2)]
    for mo in range(NMO):
        m = NSZ[mo]
        pst = ps.tile([128, 2 * L], F32, tag="acc")
        for ki in range(KD):
            nc.tensor.matmul(pst[:m, :], w.qWT[:, ki, NOFF[mo]:NOFF[mo] + m],
                             st.x2T[:, ki, :], start=(ki == 0), stop=(ki == KD - 1))
        zst = act.tile([128, 2 * L], BF16, tag="scr1", bufs=2,
                       name=f"zst_{p}_{mo}")
        if mo % 2 == 0:
            nc.scalar.activation(zst[:m, :], pst[:m, :], AF.Identity,
                                 bias=w.zbcol[:m, mo:mo + 1])
        else:
            nc.vector.tensor_scalar(out=zst[:m, :], in0=pst[:m, :],
                                    scalar1=w.zbcol[:m, mo:mo + 1],
                                    scalar2=None, op0=OP.add)
        for e in range(2):
            nc.sync.dma_start(zf[e][NOFF[mo]:NOFF[mo] + m, :],
                              zst[:m, e * L:(e + 1) * L])
    # gathers: z rows then bexp rows (gpsimd queue order matters)
    for e in range(2):
        for mo in range(NMO):
            m = NSZ[mo]
            nc.gpsimd.indirect_dma_start(
                out=st.z[e][:m, mo, :], out_offset=None, in_=zf[e][:, :],
                in_offset=bass.IndirectOffsetOnAxis(
                    ap=st.idx[e][:m, mo:mo + 1], axis=0))

    return


def _sel_part(nc, p, st, act, small, ps, w):
    """az/bz + row sums; emb/sel matmuls + drains."""
    # ---- az/bz + row sums ----
    st.az = []
    st.bz = []
    st.azT = []
    st.bzT = []
    st.rfw = []
    for e in range(2):
        az = act.tile([128, NMO, L], BF16, tag=f"az{e}", bufs=1)
        bz = act.tile([128, NMO, L], BF16, tag=f"bz{e}", bufs=1)
        sum_a = small.tile([128, NMO], F32, tag="sum_a")
        sum_b = small.tile([128, NMO], F32, tag="sum_b")
        nc.vector.memset(sum_a[:], 1.0)
        nc.vector.memset(sum_b[:], 1.0)
        for mo in range(NMO):
            m = NSZ[mo]
            nc.vector.scalar_tensor_tensor(
                out=az[:m, mo, :], in0=st.z[e][:m, mo, :], scalar=0.0,
                in1=st.mask[e][:m, mo, :], op0=OP.max, op1=OP.mult,
                accum_out=sum_a[:m, mo:mo + 1])
            nc.vector.scalar_tensor_tensor(
                out=bz[:m, mo, :], in0=st.z[e][:m, mo, :], scalar=0.0,
                in1=st.mask[e][:m, mo, :], op0=OP.min, op1=OP.mult,
                accum_out=sum_b[:m, mo:mo + 1])
        rfw_a = small.tile([128, NMO], F32, tag="rfw_a")
        rfw_b = small.tile([128, NMO], F32, tag="rfw_b")
        tmp_a = small.tile([128, NMO], F32, tag="tmp_a")
        tmp_b = small.tile([128, NMO], F32, tag="tmp_b")
        nc.vector.tensor_scalar_add(tmp_a[:], sum_a[:], EPS)
        nc.vector.reciprocal(rfw_a[:], tmp_a[:])
        nc.vector.tensor_scalar_add(tmp_b[:], sum_b[:], -EPS)
        nc.vector.reciprocal(rfw_b[:], tmp_b[:])
        st.rfw.append((rfw_a, rfw_b))

        st.az.append(az)
        st.bz.append(bz)

    # ---- emb/sel matmuls: 5 streams share each stationary x2T slice ----
    st.emb = []
    st.sel = []
    for e in range(2):
        emb_a = act.tile([128, LT, D], BF16, tag=f"emb_a{e}", bufs=1)
        emb_b = act.tile([128, LT, D], BF16, tag=f"emb_b{e}", bufs=1)
        sel = act.tile([128, LT, D], BF16, tag=f"sel{e}", bufs=1)
        for lt in range(LT):
            grp = [ps.tile([128, D], F32, tag="acc", name=f"emb_{p}_{e}_{lt}_{g}")
                   for g in range(5)]
            for ki in range(KD):
                lhs = st.x2T[:, ki, e * L + lt * 128:e * L + lt * 128 + 128]
                for g, wi in enumerate((W_A, W_GA, W_B, W_GB, W_S)):
                    nc.tensor.matmul(grp[g][:], lhs, w.w5[:, wi, ki, :],
                                     start=(ki == 0), stop=False)
            for g, bi in ((1, B_GA), (3, B_GB), (4, B_S), (0, B_A), (2, B_B)):
                nc.tensor.matmul(grp[g][:], w.ones_row[:], w.bias6[:, bi, :],
                                 start=False, stop=True)
            # drains: sigmoids first (ACT), then the DVE mults
            sigs = []
            for g_gate in (1, 3):
                sig = act.tile([128, D], BF16, tag="scr1", bufs=2,
                               name=f"sig_{p}_{e}_{lt}_{g_gate}")
                nc.scalar.activation(sig[:], grp[g_gate][:], AF.Sigmoid)
                sigs.append(sig)
            nc.scalar.activation(sel[:, lt, :], grp[4][:], AF.Sigmoid)
            for side, g_emb in enumerate((0, 2)):
                dst = emb_a if side == 0 else emb_b
                nc.vector.tensor_tensor(out=dst[:, lt, :], in0=grp[g_emb][:],
                                        in1=sigs[side][:], op=OP.mult)
        st.emb.append((emb_a, emb_b))
        st.sel.append(sel)
    return st


def _mid(nc, p, st, act, small, ps, pstr, w, x_d):
    """cfw + bw matmuls for both sides/elements, combine into y2."""
    st.y2 = []
    st.azT = []
    st.bzT = []
    st.rbw = []
    st.xr = []
    for e in range(2):
        xr = act.tile([128, LT, D], F32, tag=f"xr{e}", bufs=1,
                      name=f"xr_{p}_{e}")
        nc.sync.dma_start(xr[:], x_d[st.bs[e]].rearrange("(lt p) d -> p lt d",
                                                         p=128))
        st.xr.append(xr)
    for e in range(2):
        azT = act.tile([128, LT, NMO, 128], BF16, tag=f"azT{e}", bufs=1,
                       name=f"azT_{p}_{e}")
        bzT = act.tile([128, LT, NMO, 128], BF16, tag=f"bzT{e}", bufs=1,
                       name=f"bzT_{p}_{e}")
        for side, (zz, zzT) in enumerate(((st.az[e], azT), (st.bz[e], bzT))):
            cpeng = side
            for lt in range(LT):
                for half in range(2):
                    ptr = pstr.tile([128, 4, 128], BF16, tag="tr",
                                    name=f"trz_{p}_{e}_{side}_{lt}_{half}")
                    for mi in range(4):
                        mo = half * 4 + mi
                        m = NSZ[mo]
                        nc.tensor.transpose(
                            ptr[:, mi, :m],
                            zz[:m, mo, lt * 128:(lt + 1) * 128],
                            w.ident[:m, :m])
                    if cpeng == 0:
                        nc.vector.tensor_copy(
                            zzT[:, lt, half * 4:half * 4 + 4, :], ptr[:, :, :])
                    else:
                        nc.scalar.copy(
                            zzT[:, lt, half * 4:half * 4 + 4, :], ptr[:, :, :])
                    if half == 1:
                        nc.vector.memset(zzT[:, lt, 7, 96:128], 0.0)
        st.azT.append(azT)
        st.bzT.append(bzT)
        st.rbw.append(None)  # filled between cfw and bw emission

    for e in range(2):
        # cfw[n, d] = rfw[n] * sum_l zzT[l, n]^T emb[l, d] + bexp
        # (sides interleaved per mo so the rotating bexp tile dies early)
        cfws = [act.tile([128, NMO, D], BF16, tag=f"cfw{s}", bufs=1,
                         name=f"cfw_{p}_{e}_{s}") for s in range(2)]
        for mo in range(NMO):
            m = NSZ[mo]
            bx = act.tile([128, D], BF16, tag="bexp", bufs=2,
                          name=f"bexp_{p}_{e}_{mo}")
            nc.gpsimd.indirect_dma_start(
                out=bx[:m, :], out_offset=None, in_=w.btab_d[:, :],
                in_offset=bass.IndirectOffsetOnAxis(
                    ap=st.idx[e][:m, mo:mo + 1], axis=0))
            psts = []
            for side in range(2):
                zzT = st.azT[e] if side == 0 else st.bzT[e]
                emb = st.emb[e][side]
                pst = ps.tile([128, D], F32, tag="acc",
                              name=f"psc_{p}_{e}_{mo}_{side}")
                for lt in range(LT):
                    nc.tensor.matmul(pst[:m, :],
                                     zzT[:, lt, mo, :m],
                                     emb[:, lt, :], start=(lt == 0),
                                     stop=(lt == LT - 1))
                psts.append(pst)
            for side in range(2):
                nc.vector.scalar_tensor_tensor(
                    out=cfws[side][:m, mo, :], in0=psts[side][:m, :],
                    scalar=st.rfw[e][side][:m, mo:mo + 1],
                    in1=bx[:m, :],
                    op0=OP.mult, op1=OP.add)

        azT, bzT = st.azT[e], st.bzT[e]
        den_a = small.tile([128, LT], F32, tag="den_a")
        den_b = small.tile([128, LT], F32, tag="den_b")
        nc.vector.tensor_reduce(den_a[:], azT[:, :, :, :], axis=AX.XY, op=OP.add)
        nc.vector.tensor_reduce(den_b[:], bzT[:, :, :, :], axis=AX.XY, op=OP.add)
        rbw_a = small.tile([128, LT], F32, tag="rbw_a")
        rbw_b = small.tile([128, LT], F32, tag="rbw_b")
        t2a = small.tile([128, LT], F32, tag="t2a")
        t2b = small.tile([128, LT], F32, tag="t2b")
        nc.vector.tensor_scalar_add(t2a[:], den_a[:], EPS)
        nc.vector.reciprocal(rbw_a[:], t2a[:])
        nc.vector.tensor_scalar_add(t2b[:], den_b[:], -EPS)
        nc.vector.reciprocal(rbw_b[:], t2b[:])
        st.rbw[e] = (rbw_a, rbw_b)

        outs = []
        for side in range(2):
            zz = st.az[e] if side == 0 else st.bz[e]
            rbw = st.rbw[e][side]
            cfw = cfws[side]
            # bw: out[l, d] = rbw[l] * sum_n zz[n, l] cfw[n, d]
            out_raw = act.tile([128, LT, D], BF16, tag=f"out_{side}", bufs=1,
                               name=f"outr_{p}_{e}_{side}")
            for lt in range(LT):
                pst = ps.tile([128, D], F32, tag="acc")
                for mo in range(NMO):
                    m = NSZ[mo]
                    nc.tensor.matmul(pst[:],
                                     zz[:m, mo, lt * 128:(lt + 1) * 128],
                                     cfw[:m, mo, :], start=(mo == 0),
                                     stop=(mo == NMO - 1))
                if lt == 0:
                    nc.scalar.activation(out_raw[:, lt, :], pst[:], AF.Identity,
                                         scale=rbw[:, lt:lt + 1])
                else:
                    nc.vector.tensor_scalar(out=out_raw[:, lt, :], in0=pst[:],
                                            scalar1=rbw[:, lt:lt + 1],
                                            scalar2=None, op0=OP.mult)
            outs.append(out_raw)

        # combine: y2 = x + out_b + sel * (out_a - out_b)
        out_a, out_b = outs
        y2 = act.tile([128, LT, D], BF16, tag=f"y2_{e}", bufs=1,
                      name=f"y2_{p}_{e}")
        for lt in range(LT):
            dt_ = act.tile([128, D], BF16, tag="scr1", bufs=2)
            nc.vector.tensor_tensor(out=dt_[:], in0=out_a[:, lt, :],
                                    in1=out_b[:, lt, :], op=OP.subtract)
            nc.vector.tensor_tensor(out=dt_[:], in0=dt_[:],
                                    in1=st.sel[e][:, lt, :], op=OP.mult)
            nc.gpsimd.tensor_tensor(out=y2[:, lt, :], in0=st.xr[e][:, lt, :],
                                    in1=out_b[:, lt, :], op=OP.add)
            nc.vector.tensor_tensor(out=y2[:, lt, :], in0=y2[:, lt, :],
                                    in1=dt_[:], op=OP.add)
        st.y2.append(y2)


def _ff(nc, st, act, small, ps, pstr, w, out_d):
    """LN2 + feed-forward + residual for both elements of the pair."""
    p = st.p
    # LN2 -> x3 -> x3T (pair-interleaved)
    x3T = act.tile([128, KD, 2 * L], BF16, tag="x3T", bufs=1)
    for e in range(2):
        x3 = act.tile([128, LT, D], BF16, tag=f"x3_{e}", bufs=1)
        _layer_norm(nc, small, x3, st.y2[e], w.eps, f"ln2_{p}_{e}")
        for lt in range(LT):
            ptr = pstr.tile([128, KD, 128], BF16, tag="tr",
                            name=f"trf_{p}_{e}_{lt}")
            for ko in range(KD):
                nc.tensor.transpose(ptr[:, ko, :],
                                    x3[:, lt, ko * 128:(ko + 1) * 128],
                                    w.ident[:])
            off = e * L + lt * 128
            if lt == 0:
                nc.vector.tensor_copy(x3T[:, :, off:off + 128], ptr[:, :, :])
            else:
                nc.scalar.copy(x3T[:, :, off:off + 128], ptr[:, :, :])

    # hT (pair): relu(x3 @ Wf1 + bf1)^T : [dff-part, KF, 2L]
    hT = act.tile([128, KF, 2 * L], BF16, tag="hT", bufs=1)
    for mo in range(KF):
        pst = ps.tile([128, 2 * L], F32, tag="acc")
        for ki in range(KD):
            nc.tensor.matmul(pst[:], w.wf1[:, ki, mo * 128:(mo + 1) * 128],
                             x3T[:, ki, :], start=(ki == 0), stop=(ki == KD - 1))
        if mo % 2 == 0:
            nc.scalar.activation(hT[:, mo, :], pst[:], AF.Relu,
                                 bias=w.bf1col[:, mo:mo + 1])
        else:
            nc.vector.tensor_scalar(out=hT[:, mo, :], in0=pst[:],
                                    scalar1=w.bf1col[:, mo:mo + 1],
                                    scalar2=0.0, op0=OP.add, op1=OP.max)

    # ffout per element: out = y2 + hT^T @ Wf2 + bf2
    for e in range(2):
        osb = act.tile([128, LT, D], BF16, tag=f"osb{e}", bufs=1)
        for lt in range(LT):
            pst = ps.tile([128, D], F32, tag="acc")
            off = e * L + lt * 128
            for mo in range(KF):
                nc.tensor.matmul(pst[:], hT[:, mo, off:off + 128],
                                 w.wf2[:, mo, :], start=(mo == 0),
                                 stop=False)
            nc.tensor.matmul(pst[:], w.ones_row[:], w.bias6[:, B_F2, :],
                             start=False, stop=True)
            nc.vector.scalar_tensor_tensor(
                out=osb[:, lt, :], in0=pst[:], scalar=1.0,
                in1=st.y2[e][:, lt, :], op0=OP.mult, op1=OP.add)
        nc.sync.dma_start(out_d[st.bs[e]].rearrange("(lt p) d -> p lt d", p=128),
                          osb[:])


# ---------------------------------------------------------------------------
# host-side weight preprocessing + SPMD launch
# ---------------------------------------------------------------------------

def _prep_host(inputs):
    f = lambda k: np.ascontiguousarray(np.asarray(inputs[k], dtype=np.float32))
    g1, b1 = f("ln1_g"), f("ln1_b")
    g2, b2 = f("ln2_g"), f("ln2_b")
    Wk, bk = f("Wk"), f("bk")
    Wa, ba = f("Wa"), f("ba")
    Wa1, ba1 = f("Wa1"), f("ba1")
    Wb, bb = f("Wb"), f("bb")
    Wb1, bb1 = f("Wb1"), f("bb1")
    Ws, bsel = f("Ws"), f("bsel")
    Wf1, bf1 = f("Wf1"), f("bf1")
    Wf2, bf2 = f("Wf2"), f("bf2")
    q_tab, b_tab = f("q_tab"), f("b_tab")

    qs = q_tab / np.sqrt(np.float32(D))
    qW = qs @ (g1[:, None] * Wk).T                 # [N, D]
    zb = qs @ (b1 @ Wk + bk)                       # [N]
    qWT = np.ascontiguousarray(qW.T)               # [D, N]

    Waa1 = Wa @ Wa1
    Wbb1 = Wb @ Wb1
    w5 = np.stack([
        g1[:, None] * Wa,
        g1[:, None] * Waa1,
        g1[:, None] * Wb,
        g1[:, None] * Wbb1,
        g1[:, None] * Ws,
    ]).astype(np.float32)
    bias6 = np.stack([
        b1 @ Wa + ba,
        b1 @ Waa1 + ba @ Wa1 + ba1,
        b1 @ Wb + bb,
        b1 @ Wbb1 + bb @ Wb1 + bb1,
        b1 @ Ws + bsel,
        bf2,
    ]).astype(np.float32)                          # [6, D]
    bias6_bcast = np.broadcast_to(bias6[None, :, :], (128, 6, D))

    zb_col = np.zeros((128, NMO), np.float32)
    zb_pad = np.zeros((NPAD,), np.float32)
    zb_pad[:N] = zb
    for mo in range(NMO):
        zb_col[:, mo] = zb_pad[mo * 128:(mo + 1) * 128]

    wf1 = g2[:, None] * Wf1
    bf1p = (b2 @ Wf1 + bf1).astype(np.float32)     # [DFF]
    bf1_col = np.zeros((128, KF), np.float32)
    for mo in range(KF):
        bf1_col[:, mo] = bf1p[mo * 128:(mo + 1) * 128]

    bf = lambda a: np.ascontiguousarray(np.asarray(a, dtype=np_bf16))
    return dict(
        ident=np.ascontiguousarray(np.eye(128, dtype=np_bf16)),
        qWT=bf(qWT), w5=bf(w5), b_tab=bf(b_tab),
        bias6=bf(bias6_bcast), zbcol=np.ascontiguousarray(zb_col),
        bf1col=np.ascontiguousarray(bf1_col),
        wf1=bf(wf1), wf2=bf(Wf2),
    )


_NC_CACHE = {}


def _get_program(n_elems=BPC):
    if n_elems not in _NC_CACHE:
        _NC_CACHE[n_elems] = build_program(n_elems)
    return _NC_CACHE[n_elems]


def make_in_maps(inputs):
    x = np.ascontiguousarray(np.asarray(inputs["x"], dtype=np.float32))
    nidx = np.asarray(inputs["n_indexes"]).astype(np.int32)
    nidx_pad = np.zeros((BS, NPAD), np.int32)
    nidx_pad[:, :N] = nidx
    mask = (np.asarray(inputs["mask"]) != 0)
    mask_pad = np.zeros((BS, NPAD, L), np_bf16)
    mask_pad[:, :N, :] = mask.astype(np_bf16)
    shared = _prep_host(inputs)
    in_maps = []
    for c in range(NCORES):
        sl = slice(c * BPC, (c + 1) * BPC)
        in_maps.append({
            "x": np.ascontiguousarray(x[sl]),
            "nidx": np.ascontiguousarray(nidx_pad[sl]),
            "mask": np.ascontiguousarray(mask_pad[sl]),
            **shared,
        })
    return in_maps


def kernel(**inputs):
    from concourse.bass_utils import run_bass_kernel_spmd

    nc = _get_program(BPC)
    in_maps = make_in_maps(inputs)
    res = run_bass_kernel_spmd(nc, in_maps, core_ids=list(range(NCORES)))
    out = np.concatenate([res.results[c]["out"] for c in range(NCORES)], axis=0)
    return out.astype(np.float32)


# revision 45
# speedup vs baseline: 1.0428x; 1.0428x over previous
"""Trainium2 Bass kernel for nn_EncoderLayer (dense transformer encoder layer
with static-expansion attention-like block + FF), data-parallel over 8 cores.

Contract: kernel(**inputs) takes FULL unsharded inputs (as in setup_inputs()),
returns the FULL (64, 256, 512) float32 output.

v2: bf16 matmuls (FWL weight loads), Wk folded into q_tab on host, DMA-engine
transposes (XBAR) instead of PE transposes, biases via broadcast tiles + DVE,
bw denominators via DVE reduce, batch elements processed in pairs so the
zfull/FF matmul streams run at N=512, single merged loop (no y2 roundtrip),
software-pipelined so the FF block of pair p-1 fills the gather latency of
pair p.
"""

import sys

for _p in ("/opt/trn_rl_repo",):
    if _p not in sys.path:
        sys.path.insert(0, _p)

import numpy as np
from ml_dtypes import bfloat16 as np_bf16

import concourse.bass as bass
import concourse.mybir as mybir
import concourse.tile as tile
from concourse.vector_clock import ScopedClock

F32 = mybir.dt.float32
BF16 = mybir.dt.bfloat16
I32 = mybir.dt.int32
OP = mybir.AluOpType
AF = mybir.ActivationFunctionType
AX = mybir.AxisListType

D = 512          # d_model
DFF = 2048       # d_ff
N = 992          # n experts
NPAD = 1024
L = 256          # enc len
BS = 64
NCORES = 8
BPC = BS // NCORES  # batch elements per core
EPS = 1e-9
LN_EPS = 1e-5

KD = D // 128     # 4 k-chunks over d_model
LT = L // 128     # 2 l-chunks
NMO = 8           # n-chunks over N (7x128 + 96)
NSZ = [128] * 7 + [96]
NOFF = [128 * i for i in range(8)]
KF = DFF // 128   # 16 chunks over d_ff

# bias6 row indices
B_A, B_GA, B_B, B_GB, B_S, B_F2 = range(6)
# w5 slot indices
W_A, W_GA, W_B, W_GB, W_S = range(5)


class SplitDrainTC(tile.TileContext):
    """TileContext whose exit drain splits semaphore waits across nop
    instructions (this walrus build rejects >2 sync waits on one Drain)."""

    def _drain_and_barrier(self, tick_clock, wait_clock):
        nc = self.nc
        probe = nc.sync.nop(nofuse=True)
        wait_clock.add_sem_waits(probe.ins, ScopedClock({None: tick_clock.global_clock}))
        si = probe.ins.sync_info
        waits = list(si.on_wait) if si and si.on_wait else []
        if len(waits) > 1:
            si.on_wait = waits[:1]
            sems_by_name = {h.name: h for h in self.sems.allocated().values()}
            for w in waits[1:]:
                n2 = nc.sync.nop(nofuse=True)
                n2.wait_op(sems_by_name[w.ant_name], w.wait_value, "sem-ge")
        nc.sync.drain()
        nc.all_engine_barrier()
        popped = nc._tile_sem_poison_stack.pop()
        assert popped is self._sem_poison
        nc.clear_and_free_semaphores(list(self.sems.allocated().values()))
        nc.all_engine_barrier()


def _split_excess_waits(nc, cap=2):
    """This walrus build rejects instructions carrying more than ~2 sync
    waits. Hoist excess waits onto same-engine nop instructions inserted
    immediately before the offending instruction (engine program order is
    bb order, so the nop's waits complete first)."""
    import bass_rust
    for f in nc.m.functions:
        for bb in f.blocks:
            over = [inst for inst in bb.instructions
                    if inst.sync_info and inst.sync_info.on_wait
                    and len(inst.sync_info.on_wait) > cap]
            if not over:
                continue
            carriers = {}
            for inst in over:
                waits = list(inst.sync_info.on_wait)
                inst.sync_info.on_wait = waits[:cap]
                rest = waits[cap:]
                lst = []
                for i in range(0, len(rest), cap):
                    nop = nc.engines[inst.engine].nop(nofuse=True)
                    cur = nc.cur_bb.bb
                    assert cur.instructions[-1] is nop.ins
                    cur.instructions.pop()
                    nop.ins.sync_info = bass_rust.SyncInfo(
                        on_wait=rest[i:i + cap], on_update=[])
                    lst.append(nop.ins)
                carriers[inst.name] = lst
            out = []
            for inst in bb.instructions:
                out.extend(carriers.get(inst.name, ()))
                out.append(inst)
            bb.instructions[:] = out


def build_program(n_elems=BPC):
    """Single-core SPMD program; see kernel() for the per-core input map."""
    nc = bass.Bass("TRN2", target_bir_lowering=False, debug=False)

    x_d = nc.dram_tensor("x", [n_elems, L, D], F32, kind="ExternalInput").ap()
    nidx_d = nc.dram_tensor("nidx", [n_elems, NPAD], I32, kind="ExternalInput").ap()
    mask_d = nc.dram_tensor("mask", [n_elems, NPAD, L], BF16, kind="ExternalInput").ap()
    qWT_d = nc.dram_tensor("qWT", [D, N], BF16, kind="ExternalInput").ap()
    w5_d = nc.dram_tensor("w5", [5, D, D], BF16, kind="ExternalInput").ap()
    btab_d = nc.dram_tensor("b_tab", [N, D], BF16, kind="ExternalInput").ap()
    bias6_d = nc.dram_tensor("bias6", [6, D], BF16, kind="ExternalInput").ap()
    zbcol_d = nc.dram_tensor("zbcol", [128, NMO], F32, kind="ExternalInput").ap()
    bf1col_d = nc.dram_tensor("bf1col", [128, KF], F32, kind="ExternalInput").ap()
    wf1_d = nc.dram_tensor("wf1", [D, DFF], BF16, kind="ExternalInput").ap()
    wf2_d = nc.dram_tensor("wf2", [DFF, D], BF16, kind="ExternalInput").ap()
    xbf_d = nc.dram_tensor("xbf", [n_elems, L, D], BF16, kind="ExternalInput").ap()
    ident_d = nc.dram_tensor("ident", [128, 128], BF16, kind="ExternalInput").ap()
    out_d = nc.dram_tensor("out", [n_elems, L, D], BF16, kind="ExternalOutput").ap()

    with SplitDrainTC(nc) as tc:
        _emit(nc, tc, n_elems, x_d, nidx_d, mask_d, qWT_d, w5_d, btab_d,
              bias6_d, zbcol_d, bf1col_d, wf1_d, wf2_d, xbf_d, ident_d, out_d)
    _split_excess_waits(nc, cap=1)
    return nc


def _layer_norm(nc, pool_small, xn, x_sb, eps_tile, uid):
    """xn[:, lt, :] = (x - mean)/sqrt(var + LN_EPS)."""
    for lt in range(LT):
        stats = pool_small.tile([128, 6], F32, tag="ln_stats")
        nc.vector.bn_stats(stats[:], x_sb[:, lt, :])
        aggr = pool_small.tile([128, 2], F32, tag="ln_aggr")
        nc.vector.bn_aggr(aggr[:], stats[:])
        sv = pool_small.tile([128, 1], F32, tag="ln_sv")
        nc.scalar.activation(sv[:], aggr[:, 1:2], AF.Sqrt, bias=eps_tile[:])
        rstd = pool_small.tile([128, 1], F32, tag="ln_rstd")
        nc.vector.reciprocal(rstd[:], sv[:])
        nmr = pool_small.tile([128, 1], F32, tag="ln_nmr")
        nc.vector.tensor_scalar(out=nmr[:], in0=aggr[:, 0:1], scalar1=rstd[:],
                                scalar2=-1.0, op0=OP.mult, op1=OP.mult)
        if lt == 0:
            nc.scalar.activation(xn[:, lt, :], x_sb[:, lt, :], AF.Identity,
                                 bias=nmr[:], scale=rstd[:])
        else:
            nc.vector.tensor_scalar(out=xn[:, lt, :], in0=x_sb[:, lt, :],
                                    scalar1=rstd[:], scalar2=nmr[:],
                                    op0=OP.mult, op1=OP.add)


class _Weights:
    pass


def _emit(nc, tc, n_elems, x_d, nidx_d, mask_d, qWT_d, w5_d, btab_d,
          bias6_d, zbcol_d, bf1col_d, wf1_d, wf2_d, xbf_d, ident_d, out_d):
    from contextlib import ExitStack

    top = ExitStack()
    with top:
        w = _Weights()
        wpool = top.enter_context(tc.tile_pool(name="w", bufs=1))
        w.eps = wpool.tile([128, 1], F32)
        nc.vector.memset(w.eps[:], LN_EPS)
        w.ident = wpool.tile([128, 128], BF16)
        nc.sync.dma_start(w.ident[:], ident_d)
        w.ones_row = wpool.tile([1, 128], BF16)
        nc.vector.memset(w.ones_row[:], 1.0)
        w.btab_d = btab_d
        w.xbf_d = xbf_d

        def _load_weights():
            w.qWT = wpool.tile([128, KD, N], BF16, name="qWT_sb")
            nc.sync.dma_start(w.qWT[:], qWT_d.rearrange("(k p) n -> p k n", p=128))
            w.w5 = wpool.tile([128, 5, KD, D], BF16, name="w5_sb")
            for wi in range(5):
                nc.scalar.dma_start(w.w5[:, wi, :, :],
                                    w5_d[wi].rearrange("(k p) n -> p k n", p=128))
            w.wf1 = wpool.tile([128, KD, DFF], BF16, name="wf1_sb")
            nc.scalar.dma_start(w.wf1[:, :, :DFF // 2],
                                wf1_d[:, :DFF // 2].rearrange("(k p) n -> p k n", p=128))
            nc.scalar.dma_start(w.wf1[:, :, DFF // 2:],
                                wf1_d[:, DFF // 2:].rearrange("(k p) n -> p k n", p=128))
            w.wf2 = wpool.tile([128, KF, D], BF16, name="wf2_sb")
            nc.scalar.dma_start(w.wf2[:], wf2_d.rearrange("(k p) n -> p k n", p=128))
            w.bias6 = wpool.tile([1, 6, D], BF16, name="bias6_sb")
            nc.scalar.dma_start(w.bias6[:], bias6_d.rearrange("(o s) d -> o s d", o=1))
            w.zbcol = wpool.tile([128, NMO], F32, name="zbcol_sb")
            nc.scalar.dma_start(w.zbcol[:], zbcol_d)
            w.bf1col = wpool.tile([128, KF], F32, name="bf1col_sb")
            nc.scalar.dma_start(w.bf1col[:], bf1col_d)

        ps = top.enter_context(tc.tile_pool(name="ps", bufs=7, space="PSUM"))
        pstr = top.enter_context(tc.tile_pool(name="pstr", bufs=1, space="PSUM"))
        act = top.enter_context(tc.tile_pool(name="act", bufs=1))
        small = top.enter_context(tc.tile_pool(name="small", bufs=2))
        dram = top.enter_context(tc.tile_pool(name="dram", bufs=2, space="DRAM"))

        n_pairs = n_elems // 2
        prev = None
        nxt = _pre(nc, 0, act, small, pstr, w, x_d, nidx_d, mask_d)
        _load_weights()
        for p in range(n_pairs):
            st = nxt
            _zfull_part(nc, p, st, act, small, ps, dram, w)
            if prev is not None:
                _ff(nc, prev, act, small, ps, pstr, w, out_d)
            _sel_part(nc, p, st, act, small, ps, w)
            if p + 1 < n_pairs:
                nxt = _pre(nc, p + 1, act, small, pstr, w, x_d, nidx_d, mask_d)
            _mid(nc, p, st, act, small, ps, pstr, w, x_d)
            prev = st
        _ff(nc, prev, act, small, ps, pstr, w, out_d)


class _PairState:
    pass


def _pre(nc, p, act, small, pstr, w, x_d, nidx_d, mask_d):
    """Pair-p input loads + LN1 + x2T transposes (hoisted one stage early)."""
    st = _PairState()
    st.p = p
    b0, b1 = 2 * p, 2 * p + 1
    st.bs = (b0, b1)

    st.idx = []
    st.mask = []
    xs = []
    for e, b in enumerate(st.bs):
        xt = act.tile([128, LT, D], F32, tag=f"x{e}", bufs=1,
                      name=f"x_{p}_{e}")
        nc.sync.dma_start(xt[:], x_d[b].rearrange("(lt p) d -> p lt d", p=128))
        xs.append(xt)
        it = act.tile([128, NMO], I32, tag=f"idx{e}", bufs=2,
                      name=f"idx_{p}_{e}")
        nc.scalar.dma_start(it[:], nidx_d[b].rearrange("(a p) -> p a", p=128))
        st.idx.append(it)
        mt = act.tile([128, NMO, L], BF16, tag=f"mask{e}", bufs=1,
                      name=f"mask_{p}_{e}")
        nc.sync.dma_start(mt[:], mask_d[b].rearrange("(mo p) l -> p mo l", p=128))
        st.mask.append(mt)

    st.x2T = act.tile([128, KD, 2 * L], BF16, tag="x2T", bufs=1,
                      name=f"x2T_{p}")
    for e in range(2):
        xn = act.tile([128, LT, D], BF16, tag=f"xn{e}", bufs=1,
                      name=f"xn_{p}_{e}")
        _layer_norm(nc, small, xn, xs[e], w.eps, f"ln1_{p}_{e}")
        for lt in range(LT):
            ptr = pstr.tile([128, KD, 128], BF16, tag="tr",
                            name=f"trx_{p}_{e}_{lt}")
            for ko in range(KD):
                nc.tensor.transpose(ptr[:, ko, :],
                                    xn[:, lt, ko * 128:(ko + 1) * 128],
                                    w.ident[:])
            off = e * L + lt * 128
            if lt == 0:
                nc.scalar.copy(st.x2T[:, :, off:off + 128], ptr[:, :, :])
            else:
                nc.vector.tensor_copy(st.x2T[:, :, off:off + 128], ptr[:, :, :])
    return st


def _zfull_part(nc, p, st, act, small, ps, dram, w):
    """zfull matmuls + stores + z gathers."""
    # ---- zfull (pair): z[n, 2L] = qW @ xn^T + zb ----
    zf = [dram.tile([N, L], BF16, tag=f"zfull{e}", bufs=2, name=f"zf_{p}_{e}")
          for e in range(2)]
    st.z = [act.tile([128, NMO, L], BF16, tag=f"z{e}", bufs=1,
                     name=f"z_{p}_{e}") for e in range(2)]
    for mo in range(NMO):
        m = NSZ[mo]
        pst = ps.tile([128, 2 * L], F32, tag="acc")
        for ki in range(KD):
            nc.tensor.matmul(pst[:m, :], w.qWT[:, ki, NOFF[mo]:NOFF[mo] + m],
                             st.x2T[:, ki, :], start=(ki == 0), stop=(ki == KD - 1))
        zst = act.tile([128, 2 * L], BF16, tag="scr1", bufs=2,
                       name=f"zst_{p}_{mo}")
        if mo % 2 == 0:
            nc.scalar.activation(zst[:m, :], pst[:m, :], AF.Identity,
                                 bias=w.zbcol[:m, mo:mo + 1])
        else:
            nc.vector.tensor_scalar(out=zst[:m, :], in0=pst[:m, :],
                                    scalar1=w.zbcol[:m, mo:mo + 1],
                                    scalar2=None, op0=OP.add)
        for e in range(2):
            nc.sync.dma_start(zf[e][NOFF[mo]:NOFF[mo] + m, :],
                              zst[:m, e * L:(e + 1) * L])
    # gathers: z rows then bexp rows (gpsimd queue order matters)
    for e in range(2):
        for mo in range(NMO):
            m = NSZ[mo]
            nc.gpsimd.indirect_dma_start(
                out=st.z[e][:m, mo, :], out_offset=None, in_=zf[e][:, :],
                in_offset=bass.IndirectOffsetOnAxis(
                    ap=st.idx[e][:m, mo:mo + 1], axis=0))

    return


def _sel_part(nc, p, st, act, small, ps, w):
    """az/bz + row sums; emb/sel matmuls + drains."""
    # ---- az/bz + row sums ----
    st.az = []
    st.bz = []
    st.azT = []
    st.bzT = []
    st.rfw = []
    for e in range(2):
        az = act.tile([128, NMO, L], BF16, tag=f"az{e}", bufs=1)
        bz = act.tile([128, NMO, L], BF16, tag=f"bz{e}", bufs=1)
        sum_a = small.tile([128, NMO], F32, tag="sum_a")
        sum_b = small.tile([128, NMO], F32, tag="sum_b")
        nc.vector.memset(sum_a[:], 1.0)
        nc.vector.memset(sum_b[:], 1.0)
        for mo in range(NMO):
            m = NSZ[mo]
            nc.vector.scalar_tensor_tensor(
                out=az[:m, mo, :], in0=st.z[e][:m, mo, :], scalar=0.0,
                in1=st.mask[e][:m, mo, :], op0=OP.max, op1=OP.mult,
                accum_out=sum_a[:m, mo:mo + 1])
            nc.vector.scalar_tensor_tensor(
                out=bz[:m, mo, :], in0=st.z[e][:m, mo, :], scalar=0.0,
                in1=st.mask[e][:m, mo, :], op0=OP.min, op1=OP.mult,
                accum_out=sum_b[:m, mo:mo + 1])
        rfw_a = small.tile([128, NMO], F32, tag="rfw_a")
        rfw_b = small.tile([128, NMO], F32, tag="rfw_b")
        tmp_a = small.tile([128, NMO], F32, tag="tmp_a")
        tmp_b = small.tile([128, NMO], F32, tag="tmp_b")
        nc.vector.tensor_scalar_add(tmp_a[:], sum_a[:], EPS)
        nc.vector.reciprocal(rfw_a[:], tmp_a[:])
        nc.vector.tensor_scalar_add(tmp_b[:], sum_b[:], -EPS)
        nc.vector.reciprocal(rfw_b[:], tmp_b[:])
        st.rfw.append((rfw_a, rfw_b))

        st.az.append(az)
        st.bz.append(bz)

    # ---- emb/sel matmuls: 5 streams share each stationary x2T slice ----
    st.emb = []
    st.sel = []
    for e in range(2):
        emb_a = act.tile([128, LT, D], BF16, tag=f"emb_a{e}", bufs=1)
        emb_b = act.tile([128, LT, D], BF16, tag=f"emb_b{e}", bufs=1)
        sel = act.tile([128, LT, D], BF16, tag=f"sel{e}", bufs=1)
        for lt in range(LT):
            grp = [ps.tile([128, D], F32, tag="acc", name=f"emb_{p}_{e}_{lt}_{g}")
                   for g in range(5)]
            for ki in range(KD):
                lhs = st.x2T[:, ki, e * L + lt * 128:e * L + lt * 128 + 128]
                for g, wi in enumerate((W_A, W_GA, W_B, W_GB, W_S)):
                    nc.tensor.matmul(grp[g][:], lhs, w.w5[:, wi, ki, :],
                                     start=(ki == 0), stop=False)
            for g, bi in ((1, B_GA), (3, B_GB), (4, B_S), (0, B_A), (2, B_B)):
                nc.tensor.matmul(grp[g][:], w.ones_row[:], w.bias6[:, bi, :],
                                 start=False, stop=True)
            # drains: sigmoids first (ACT), then the DVE mults
            sigs = []
            for g_gate in (1, 3):
                sig = act.tile([128, D], BF16, tag="scr1", bufs=2,
                               name=f"sig_{p}_{e}_{lt}_{g_gate}")
                nc.scalar.activation(sig[:], grp[g_gate][:], AF.Sigmoid)
                sigs.append(sig)
            nc.scalar.activation(sel[:, lt, :], grp[4][:], AF.Sigmoid)
            for side, g_emb in enumerate((0, 2)):
                dst = emb_a if side == 0 else emb_b
                nc.vector.tensor_tensor(out=dst[:, lt, :], in0=grp[g_emb][:],
                                        in1=sigs[side][:], op=OP.mult)
        st.emb.append((emb_a, emb_b))
        st.sel.append(sel)
    return st


def _mid(nc, p, st, act, small, ps, pstr, w, x_d):
    """cfw + bw matmuls for both sides/elements, combine into y2."""
    st.y2 = []
    st.azT = []
    st.bzT = []
    st.rbw = []
    st.xr = []
    for e in range(2):
        xr = act.tile([128, LT, D], BF16, tag=f"xr{e}", bufs=1,
                      name=f"xr_{p}_{e}")
        nc.sync.dma_start(xr[:], w.xbf_d[st.bs[e]].rearrange("(lt p) d -> p lt d",
                                                             p=128))
        st.xr.append(xr)
    for e in range(2):
        azT = act.tile([128, LT, NMO, 128], BF16, tag=f"azT{e}", bufs=1,
                       name=f"azT_{p}_{e}")
        bzT = act.tile([128, LT, NMO, 128], BF16, tag=f"bzT{e}", bufs=1,
                       name=f"bzT_{p}_{e}")
        for side, (zz, zzT) in enumerate(((st.az[e], azT), (st.bz[e], bzT))):
            cpeng = side
            for lt in range(LT):
                for half in range(2):
                    ptr = pstr.tile([128, 4, 128], BF16, tag="tr",
                                    name=f"trz_{p}_{e}_{side}_{lt}_{half}")
                    for mi in range(4):
                        mo = half * 4 + mi
                        m = NSZ[mo]
                        nc.tensor.transpose(
                            ptr[:, mi, :m],
                            zz[:m, mo, lt * 128:(lt + 1) * 128],
                            w.ident[:m, :m])
                    if cpeng == 0:
                        nc.vector.tensor_copy(
                            zzT[:, lt, half * 4:half * 4 + 4, :], ptr[:, :, :])
                    else:
                        nc.scalar.copy(
                            zzT[:, lt, half * 4:half * 4 + 4, :], ptr[:, :, :])
                    if half == 1:
                        nc.vector.memset(zzT[:, lt, 7, 96:128], 0.0)
        st.azT.append(azT)
        st.bzT.append(bzT)
        st.rbw.append(None)  # filled between cfw and bw emission

    for e in range(2):
        # cfw[n, d] = rfw[n] * sum_l zzT[l, n]^T emb[l, d] + bexp
        # (sides interleaved per mo so the rotating bexp tile dies early)
        cfws = [act.tile([128, NMO, D], BF16, tag=f"cfw{s}", bufs=1,
                         name=f"cfw_{p}_{e}_{s}") for s in range(2)]
        for mo in range(NMO):
            m = NSZ[mo]
            bx = act.tile([128, D], BF16, tag="bexp", bufs=4,
                          name=f"bexp_{p}_{e}_{mo}")
            nc.gpsimd.indirect_dma_start(
                out=bx[:m, :], out_offset=None, in_=w.btab_d[:, :],
                in_offset=bass.IndirectOffsetOnAxis(
                    ap=st.idx[e][:m, mo:mo + 1], axis=0))
            psts = []
            for side in range(2):
                zzT = st.azT[e] if side == 0 else st.bzT[e]
                emb = st.emb[e][side]
                pst = ps.tile([128, D], F32, tag="acc",
                              name=f"psc_{p}_{e}_{mo}_{side}")
                for lt in range(LT):
                    nc.tensor.matmul(pst[:m, :],
                                     zzT[:, lt, mo, :m],
                                     emb[:, lt, :], start=(lt == 0),
                                     stop=(lt == LT - 1))
                psts.append(pst)
            for side in range(2):
                nc.vector.scalar_tensor_tensor(
                    out=cfws[side][:m, mo, :], in0=psts[side][:m, :],
                    scalar=st.rfw[e][side][:m, mo:mo + 1],
                    in1=bx[:m, :],
                    op0=OP.mult, op1=OP.add)

        azT, bzT = st.azT[e], st.bzT[e]
        den_a = small.tile([128, LT], F32, tag="den_a")
        den_b = small.tile([128, LT], F32, tag="den_b")
        nc.vector.tensor_reduce(den_a[:], azT[:, :, :, :], axis=AX.XY, op=OP.add)
        nc.vector.tensor_reduce(den_b[:], bzT[:, :, :, :], axis=AX.XY, op=OP.add)
        rbw_a = small.tile([128, LT], F32, tag="rbw_a")
        rbw_b = small.tile([128, LT], F32, tag="rbw_b")
        t2a = small.tile([128, LT], F32, tag="t2a")
        t2b = small.tile([128, LT], F32, tag="t2b")
        nc.vector.tensor_scalar_add(t2a[:], den_a[:], EPS)
        nc.vector.reciprocal(rbw_a[:], t2a[:])
        nc.vector.tensor_scalar_add(t2b[:], den_b[:], -EPS)
        nc.vector.reciprocal(rbw_b[:], t2b[:])
        st.rbw[e] = (rbw_a, rbw_b)

        outs = []
        for side in range(2):
            zz = st.az[e] if side == 0 else st.bz[e]
            rbw = st.rbw[e][side]
            cfw = cfws[side]
            # bw: out[l, d] = rbw[l] * sum_n zz[n, l] cfw[n, d]
            out_raw = act.tile([128, LT, D], BF16, tag=f"out_{side}", bufs=1,
                               name=f"outr_{p}_{e}_{side}")
            for lt in range(LT):
                pst = ps.tile([128, D], F32, tag="acc")
                for mo in range(NMO):
                    m = NSZ[mo]
                    nc.tensor.matmul(pst[:],
                                     zz[:m, mo, lt * 128:(lt + 1) * 128],
                                     cfw[:m, mo, :], start=(mo == 0),
                                     stop=(mo == NMO - 1))
                if lt == 0:
                    nc.scalar.activation(out_raw[:, lt, :], pst[:], AF.Identity,
                                         scale=rbw[:, lt:lt + 1])
                else:
                    nc.vector.tensor_scalar(out=out_raw[:, lt, :], in0=pst[:],
                                            scalar1=rbw[:, lt:lt + 1],
                                            scalar2=None, op0=OP.mult)
            outs.append(out_raw)

        # combine: y2 = x + out_b + sel * (out_a - out_b)
        out_a, out_b = outs
        y2 = act.tile([128, LT, D], BF16, tag=f"y2_{e}", bufs=1,
                      name=f"y2_{p}_{e}")
        for lt in range(LT):
            dt_ = act.tile([128, D], BF16, tag="scr1", bufs=2)
            nc.vector.tensor_tensor(out=dt_[:], in0=out_a[:, lt, :],
                                    in1=out_b[:, lt, :], op=OP.subtract)
            nc.vector.tensor_tensor(out=dt_[:], in0=dt_[:],
                                    in1=st.sel[e][:, lt, :], op=OP.mult)
            nc.gpsimd.tensor_tensor(out=y2[:, lt, :], in0=st.xr[e][:, lt, :],
                                    in1=out_b[:, lt, :], op=OP.add)
            nc.vector.tensor_tensor(out=y2[:, lt, :], in0=y2[:, lt, :],
                                    in1=dt_[:], op=OP.add)
        st.y2.append(y2)


def _ff(nc, st, act, small, ps, pstr, w, out_d):
    """LN2 + feed-forward + residual for both elements of the pair."""
    p = st.p
    # LN2 -> x3 -> x3T (pair-interleaved)
    x3T = act.tile([128, KD, 2 * L], BF16, tag="x3T", bufs=1)
    for e in range(2):
        x3 = act.tile([128, LT, D], BF16, tag=f"x3_{e}", bufs=1)
        _layer_norm(nc, small, x3, st.y2[e], w.eps, f"ln2_{p}_{e}")
        for lt in range(LT):
            ptr = pstr.tile([128, KD, 128], BF16, tag="tr",
                            name=f"trf_{p}_{e}_{lt}")
            for ko in range(KD):
                nc.tensor.transpose(ptr[:, ko, :],
                                    x3[:, lt, ko * 128:(ko + 1) * 128],
                                    w.ident[:])
            off = e * L + lt * 128
            if lt == 0:
                nc.vector.tensor_copy(x3T[:, :, off:off + 128], ptr[:, :, :])
            else:
                nc.scalar.copy(x3T[:, :, off:off + 128], ptr[:, :, :])

    # hT (pair): relu(x3 @ Wf1 + bf1)^T : [dff-part, KF, 2L]
    hT = act.tile([128, KF, 2 * L], BF16, tag="hT", bufs=1)
    for mo in range(KF):
        pst = ps.tile([128, 2 * L], F32, tag="acc")
        for ki in range(KD):
            nc.tensor.matmul(pst[:], w.wf1[:, ki, mo * 128:(mo + 1) * 128],
                             x3T[:, ki, :], start=(ki == 0), stop=(ki == KD - 1))
        if mo % 2 == 0:
            nc.scalar.activation(hT[:, mo, :], pst[:], AF.Relu,
                                 bias=w.bf1col[:, mo:mo + 1])
        else:
            nc.vector.tensor_scalar(out=hT[:, mo, :], in0=pst[:],
                                    scalar1=w.bf1col[:, mo:mo + 1],
                                    scalar2=0.0, op0=OP.add, op1=OP.max)

    # ffout per element: out = y2 + hT^T @ Wf2 + bf2
    for e in range(2):
        osb = act.tile([128, LT, D], BF16, tag=f"osb{e}", bufs=1)
        for lt in range(LT):
            pst = ps.tile([128, D], F32, tag="acc")
            off = e * L + lt * 128
            for mo in range(KF):
                nc.tensor.matmul(pst[:], hT[:, mo, off:off + 128],
                                 w.wf2[:, mo, :], start=(mo == 0),
                                 stop=False)
            nc.tensor.matmul(pst[:], w.ones_row[:], w.bias6[:, B_F2, :],
                             start=False, stop=True)
            nc.vector.scalar_tensor_tensor(
                out=osb[:, lt, :], in0=pst[:], scalar=1.0,
                in1=st.y2[e][:, lt, :], op0=OP.mult, op1=OP.add)
        nc.sync.dma_start(out_d[st.bs[e]].rearrange("(lt p) d -> p lt d", p=128),
                          osb[:])


# ---------------------------------------------------------------------------
# host-side weight preprocessing + SPMD launch
# ---------------------------------------------------------------------------

def _prep_host(inputs):
    f = lambda k: np.ascontiguousarray(np.asarray(inputs[k], dtype=np.float32))
    g1, b1 = f("ln1_g"), f("ln1_b")
    g2, b2 = f("ln2_g"), f("ln2_b")
    Wk, bk = f("Wk"), f("bk")
    Wa, ba = f("Wa"), f("ba")
    Wa1, ba1 = f("Wa1"), f("ba1")
    Wb, bb = f("Wb"), f("bb")
    Wb1, bb1 = f("Wb1"), f("bb1")
    Ws, bsel = f("Ws"), f("bsel")
    Wf1, bf1 = f("Wf1"), f("bf1")
    Wf2, bf2 = f("Wf2"), f("bf2")
    q_tab, b_tab = f("q_tab"), f("b_tab")

    qs = q_tab / np.sqrt(np.float32(D))
    qW = qs @ (g1[:, None] * Wk).T                 # [N, D]
    zb = qs @ (b1 @ Wk + bk)                       # [N]
    qWT = np.ascontiguousarray(qW.T)               # [D, N]

    Waa1 = Wa @ Wa1
    Wbb1 = Wb @ Wb1
    w5 = np.stack([
        g1[:, None] * Wa,
        g1[:, None] * Waa1,
        g1[:, None] * Wb,
        g1[:, None] * Wbb1,
        g1[:, None] * Ws,
    ]).astype(np.float32)
    bias6 = np.stack([
        b1 @ Wa + ba,
        b1 @ Waa1 + ba @ Wa1 + ba1,
        b1 @ Wb + bb,
        b1 @ Wbb1 + bb @ Wb1 + bb1,
        b1 @ Ws + bsel,
        bf2,
    ]).astype(np.float32)                          # [6, D]
    bias6_bcast = np.broadcast_to(bias6[None, :, :], (128, 6, D))

    zb_col = np.zeros((128, NMO), np.float32)
    zb_pad = np.zeros((NPAD,), np.float32)
    zb_pad[:N] = zb
    for mo in range(NMO):
        zb_col[:, mo] = zb_pad[mo * 128:(mo + 1) * 128]

    wf1 = g2[:, None] * Wf1
    bf1p = (b2 @ Wf1 + bf1).astype(np.float32)     # [DFF]
    bf1_col = np.zeros((128, KF), np.float32)
    for mo in range(KF):
        bf1_col[:, mo] = bf1p[mo * 128:(mo + 1) * 128]

    bf = lambda a: np.ascontiguousarray(np.asarray(a, dtype=np_bf16))
    return dict(
        ident=np.ascontiguousarray(np.eye(128, dtype=np_bf16)),
        qWT=bf(qWT), w5=bf(w5), b_tab=bf(b_tab),
        bias6=bf(bias6_bcast), zbcol=np.ascontiguousarray(zb_col),
        bf1col=np.ascontiguousarray(bf1_col),
        wf1=bf(wf1), wf2=bf(Wf2),
    )


_NC_CACHE = {}


def _get_program(n_elems=BPC):
    if n_elems not in _NC_CACHE:
        _NC_CACHE[n_elems] = build_program(n_elems)
    return _NC_CACHE[n_elems]


def make_in_maps(inputs):
    x = np.ascontiguousarray(np.asarray(inputs["x"], dtype=np.float32))
    nidx = np.asarray(inputs["n_indexes"]).astype(np.int32)
    nidx_pad = np.zeros((BS, NPAD), np.int32)
    nidx_pad[:, :N] = nidx
    mask = (np.asarray(inputs["mask"]) != 0)
    mask_pad = np.zeros((BS, NPAD, L), np_bf16)
    mask_pad[:, :N, :] = mask.astype(np_bf16)
    shared = _prep_host(inputs)
    in_maps = []
    for c in range(NCORES):
        sl = slice(c * BPC, (c + 1) * BPC)
        in_maps.append({
            "x": np.ascontiguousarray(x[sl]),
            "xbf": np.ascontiguousarray(x[sl].astype(np_bf16)),
            "nidx": np.ascontiguousarray(nidx_pad[sl]),
            "mask": np.ascontiguousarray(mask_pad[sl]),
            **shared,
        })
    return in_maps


def kernel(**inputs):
    from concourse.bass_utils import run_bass_kernel_spmd

    nc = _get_program(BPC)
    in_maps = make_in_maps(inputs)
    res = run_bass_kernel_spmd(nc, in_maps, core_ids=list(range(NCORES)))
    out = np.concatenate([res.results[c]["out"] for c in range(NCORES)], axis=0)
    return out.astype(np.float32)


# revision 46
# speedup vs baseline: 1.0654x; 1.0217x over previous
"""Trainium2 Bass kernel for nn_EncoderLayer (dense transformer encoder layer
with static-expansion attention-like block + FF), data-parallel over 8 cores.

Contract: kernel(**inputs) takes FULL unsharded inputs (as in setup_inputs()),
returns the FULL (64, 256, 512) float32 output.

v2: bf16 matmuls (FWL weight loads), Wk folded into q_tab on host, DMA-engine
transposes (XBAR) instead of PE transposes, biases via broadcast tiles + DVE,
bw denominators via DVE reduce, batch elements processed in pairs so the
zfull/FF matmul streams run at N=512, single merged loop (no y2 roundtrip),
software-pipelined so the FF block of pair p-1 fills the gather latency of
pair p.
"""

import sys

for _p in ("/opt/trn_rl_repo",):
    if _p not in sys.path:
        sys.path.insert(0, _p)

import numpy as np
from ml_dtypes import bfloat16 as np_bf16

import concourse.bass as bass
import concourse.mybir as mybir
import concourse.tile as tile
from concourse.vector_clock import ScopedClock

F32 = mybir.dt.float32
BF16 = mybir.dt.bfloat16
I32 = mybir.dt.int32
OP = mybir.AluOpType
AF = mybir.ActivationFunctionType
AX = mybir.AxisListType

D = 512          # d_model
DFF = 2048       # d_ff
N = 992          # n experts
NPAD = 1024
L = 256          # enc len
BS = 64
NCORES = 8
BPC = BS // NCORES  # batch elements per core
EPS = 1e-9
LN_EPS = 1e-5

KD = D // 128     # 4 k-chunks over d_model
LT = L // 128     # 2 l-chunks
NMO = 8           # n-chunks over N (7x128 + 96)
NSZ = [128] * 7 + [96]
NOFF = [128 * i for i in range(8)]
KF = DFF // 128   # 16 chunks over d_ff

# bias6 row indices
B_A, B_GA, B_B, B_GB, B_S, B_F2 = range(6)
# w5 slot indices
W_A, W_GA, W_B, W_GB, W_S = range(5)


class SplitDrainTC(tile.TileContext):
    """TileContext whose exit drain splits semaphore waits across nop
    instructions (this walrus build rejects >2 sync waits on one Drain)."""

    def _drain_and_barrier(self, tick_clock, wait_clock):
        nc = self.nc
        probe = nc.sync.nop(nofuse=True)
        wait_clock.add_sem_waits(probe.ins, ScopedClock({None: tick_clock.global_clock}))
        si = probe.ins.sync_info
        waits = list(si.on_wait) if si and si.on_wait else []
        if len(waits) > 1:
            si.on_wait = waits[:1]
            sems_by_name = {h.name: h for h in self.sems.allocated().values()}
            for w in waits[1:]:
                n2 = nc.sync.nop(nofuse=True)
                n2.wait_op(sems_by_name[w.ant_name], w.wait_value, "sem-ge")
        nc.sync.drain()
        nc.all_engine_barrier()
        popped = nc._tile_sem_poison_stack.pop()
        assert popped is self._sem_poison
        nc.clear_and_free_semaphores(list(self.sems.allocated().values()))
        nc.all_engine_barrier()


def _split_excess_waits(nc, cap=2):
    """This walrus build rejects instructions carrying more than ~2 sync
    waits. Hoist excess waits onto same-engine nop instructions inserted
    immediately before the offending instruction (engine program order is
    bb order, so the nop's waits complete first)."""
    import bass_rust
    for f in nc.m.functions:
        for bb in f.blocks:
            over = [inst for inst in bb.instructions
                    if inst.sync_info and inst.sync_info.on_wait
                    and len(inst.sync_info.on_wait) > cap]
            if not over:
                continue
            carriers = {}
            for inst in over:
                waits = list(inst.sync_info.on_wait)
                inst.sync_info.on_wait = waits[:cap]
                rest = waits[cap:]
                lst = []
                for i in range(0, len(rest), cap):
                    nop = nc.engines[inst.engine].nop(nofuse=True)
                    cur = nc.cur_bb.bb
                    assert cur.instructions[-1] is nop.ins
                    cur.instructions.pop()
                    nop.ins.sync_info = bass_rust.SyncInfo(
                        on_wait=rest[i:i + cap], on_update=[])
                    lst.append(nop.ins)
                carriers[inst.name] = lst
            out = []
            for inst in bb.instructions:
                out.extend(carriers.get(inst.name, ()))
                out.append(inst)
            bb.instructions[:] = out


def build_program(n_elems=BPC):
    """Single-core SPMD program; see kernel() for the per-core input map."""
    nc = bass.Bass("TRN2", target_bir_lowering=False, debug=False)

    x_d = nc.dram_tensor("x", [n_elems, L, D], F32, kind="ExternalInput").ap()
    nidx_d = nc.dram_tensor("nidx", [n_elems, NPAD], I32, kind="ExternalInput").ap()
    mask_d = nc.dram_tensor("mask", [n_elems, NPAD, L], BF16, kind="ExternalInput").ap()
    qWT_d = nc.dram_tensor("qWT", [D, N], BF16, kind="ExternalInput").ap()
    w5_d = nc.dram_tensor("w5", [5, D, D], BF16, kind="ExternalInput").ap()
    btab_d = nc.dram_tensor("b_tab", [N, D], BF16, kind="ExternalInput").ap()
    bias6_d = nc.dram_tensor("bias6", [6, D], BF16, kind="ExternalInput").ap()
    zbcol_d = nc.dram_tensor("zbcol", [128, NMO], F32, kind="ExternalInput").ap()
    bf1col_d = nc.dram_tensor("bf1col", [128, KF], F32, kind="ExternalInput").ap()
    wf1_d = nc.dram_tensor("wf1", [D, DFF], BF16, kind="ExternalInput").ap()
    wf2_d = nc.dram_tensor("wf2", [DFF, D], BF16, kind="ExternalInput").ap()
    xbf_d = nc.dram_tensor("xbf", [n_elems, L, D], BF16, kind="ExternalInput").ap()
    ident_d = nc.dram_tensor("ident", [128, 128], BF16, kind="ExternalInput").ap()
    out_d = nc.dram_tensor("out", [n_elems, L, D], BF16, kind="ExternalOutput").ap()

    with SplitDrainTC(nc) as tc:
        _emit(nc, tc, n_elems, x_d, nidx_d, mask_d, qWT_d, w5_d, btab_d,
              bias6_d, zbcol_d, bf1col_d, wf1_d, wf2_d, xbf_d, ident_d, out_d)
    _split_excess_waits(nc, cap=1)
    return nc


def _layer_norm(nc, pool_small, xn, x_sb, eps_tile, uid):
    """xn[:, lt, :] = (x - mean)/sqrt(var + LN_EPS)."""
    for lt in range(LT):
        stats = pool_small.tile([128, 6], F32, tag="ln_stats")
        nc.vector.bn_stats(stats[:], x_sb[:, lt, :])
        aggr = pool_small.tile([128, 2], F32, tag="ln_aggr")
        nc.vector.bn_aggr(aggr[:], stats[:])
        sv = pool_small.tile([128, 1], F32, tag="ln_sv")
        nc.scalar.activation(sv[:], aggr[:, 1:2], AF.Sqrt, bias=eps_tile[:])
        rstd = pool_small.tile([128, 1], F32, tag="ln_rstd")
        nc.vector.reciprocal(rstd[:], sv[:])
        nmr = pool_small.tile([128, 1], F32, tag="ln_nmr")
        nc.vector.tensor_scalar(out=nmr[:], in0=aggr[:, 0:1], scalar1=rstd[:],
                                scalar2=-1.0, op0=OP.mult, op1=OP.mult)
        if lt == 0:
            nc.scalar.activation(xn[:, lt, :], x_sb[:, lt, :], AF.Identity,
                                 bias=nmr[:], scale=rstd[:])
        else:
            nc.vector.tensor_scalar(out=xn[:, lt, :], in0=x_sb[:, lt, :],
                                    scalar1=rstd[:], scalar2=nmr[:],
                                    op0=OP.mult, op1=OP.add)


class _Weights:
    pass


def _emit(nc, tc, n_elems, x_d, nidx_d, mask_d, qWT_d, w5_d, btab_d,
          bias6_d, zbcol_d, bf1col_d, wf1_d, wf2_d, xbf_d, ident_d, out_d):
    from contextlib import ExitStack

    top = ExitStack()
    with top:
        w = _Weights()
        wpool = top.enter_context(tc.tile_pool(name="w", bufs=1))
        w.eps = wpool.tile([128, 1], F32)
        nc.vector.memset(w.eps[:], LN_EPS)
        w.ident = wpool.tile([128, 128], BF16)
        nc.sync.dma_start(w.ident[:], ident_d)
        w.ones_row = wpool.tile([1, 128], BF16)
        nc.vector.memset(w.ones_row[:], 1.0)
        w.btab_d = btab_d
        w.xbf_d = xbf_d

        def _load_weights():
            w.qWT = wpool.tile([128, KD, N], BF16, name="qWT_sb")
            nc.sync.dma_start(w.qWT[:], qWT_d.rearrange("(k p) n -> p k n", p=128))
            w.w5 = wpool.tile([128, 5, KD, D], BF16, name="w5_sb")
            for wi in range(5):
                nc.scalar.dma_start(w.w5[:, wi, :, :],
                                    w5_d[wi].rearrange("(k p) n -> p k n", p=128))
            w.wf1 = wpool.tile([128, KD, DFF], BF16, name="wf1_sb")
            nc.scalar.dma_start(w.wf1[:, :, :DFF // 2],
                                wf1_d[:, :DFF // 2].rearrange("(k p) n -> p k n", p=128))
            nc.scalar.dma_start(w.wf1[:, :, DFF // 2:],
                                wf1_d[:, DFF // 2:].rearrange("(k p) n -> p k n", p=128))
            w.wf2 = wpool.tile([128, KF, D], BF16, name="wf2_sb")
            nc.scalar.dma_start(w.wf2[:], wf2_d.rearrange("(k p) n -> p k n", p=128))
            w.bias6 = wpool.tile([1, 6, D], BF16, name="bias6_sb")
            nc.scalar.dma_start(w.bias6[:], bias6_d.rearrange("(o s) d -> o s d", o=1))
            w.zbcol = wpool.tile([128, NMO], F32, name="zbcol_sb")
            nc.scalar.dma_start(w.zbcol[:], zbcol_d)
            w.bf1col = wpool.tile([128, KF], F32, name="bf1col_sb")
            nc.scalar.dma_start(w.bf1col[:], bf1col_d)

        ps = top.enter_context(tc.tile_pool(name="ps", bufs=7, space="PSUM"))
        pstr = top.enter_context(tc.tile_pool(name="pstr", bufs=1, space="PSUM"))
        act = top.enter_context(tc.tile_pool(name="act", bufs=1))
        small = top.enter_context(tc.tile_pool(name="small", bufs=2))
        dram = top.enter_context(tc.tile_pool(name="dram", bufs=2, space="DRAM"))

        n_pairs = n_elems // 2
        prev = None
        nxt = _pre(nc, 0, act, small, pstr, w, x_d, nidx_d, mask_d)
        _load_weights()
        for p in range(n_pairs):
            st = nxt
            _zfull_part(nc, p, st, act, small, ps, dram, w)
            if prev is not None:
                _ff(nc, prev, act, small, ps, pstr, w, out_d)
            _sel_part(nc, p, st, act, small, ps, w)
            if p + 1 < n_pairs:
                nxt = _pre(nc, p + 1, act, small, pstr, w, x_d, nidx_d, mask_d)
            _mid(nc, p, st, act, small, ps, pstr, w, x_d)
            prev = st
        _ff(nc, prev, act, small, ps, pstr, w, out_d)


class _PairState:
    pass


def _pre(nc, p, act, small, pstr, w, x_d, nidx_d, mask_d):
    """Pair-p input loads + LN1 + x2T transposes (hoisted one stage early)."""
    st = _PairState()
    st.p = p
    b0, b1 = 2 * p, 2 * p + 1
    st.bs = (b0, b1)

    st.idx = []
    st.mask = []
    xs = []
    for e, b in enumerate(st.bs):
        xt = act.tile([128, LT, D], F32, tag=f"x{e}", bufs=1,
                      name=f"x_{p}_{e}")
        nc.sync.dma_start(xt[:], x_d[b].rearrange("(lt p) d -> p lt d", p=128))
        xs.append(xt)
        it = act.tile([128, NMO], I32, tag=f"idx{e}", bufs=2,
                      name=f"idx_{p}_{e}")
        nc.scalar.dma_start(it[:], nidx_d[b].rearrange("(a p) -> p a", p=128))
        st.idx.append(it)
        mt = act.tile([128, NMO, L], BF16, tag=f"mask{e}", bufs=1,
                      name=f"mask_{p}_{e}")
        nc.sync.dma_start(mt[:], mask_d[b].rearrange("(mo p) l -> p mo l", p=128))
        st.mask.append(mt)

    st.x2T = act.tile([128, KD, 2 * L], BF16, tag="x2T", bufs=1,
                      name=f"x2T_{p}")
    for e in range(2):
        xn = act.tile([128, LT, D], BF16, tag=f"xn{e}", bufs=1,
                      name=f"xn_{p}_{e}")
        _layer_norm(nc, small, xn, xs[e], w.eps, f"ln1_{p}_{e}")
        for lt in range(LT):
            ptr = pstr.tile([128, KD, 128], BF16, tag="tr",
                            name=f"trx_{p}_{e}_{lt}")
            for ko in range(KD):
                nc.tensor.transpose(ptr[:, ko, :],
                                    xn[:, lt, ko * 128:(ko + 1) * 128],
                                    w.ident[:])
            off = e * L + lt * 128
            if lt == 0:
                nc.scalar.copy(st.x2T[:, :, off:off + 128], ptr[:, :, :])
            else:
                nc.vector.tensor_copy(st.x2T[:, :, off:off + 128], ptr[:, :, :])
    return st


def _zfull_part(nc, p, st, act, small, ps, dram, w):
    """zfull matmuls + stores + z gathers."""
    # ---- zfull (pair): z[n, 2L] = qW @ xn^T + zb ----
    zf = [dram.tile([N, L], BF16, tag=f"zfull{e}", bufs=2, name=f"zf_{p}_{e}")
          for e in range(2)]
    st.z = [act.tile([128, NMO, L], BF16, tag=f"z{e}", bufs=1,
                     name=f"z_{p}_{e}") for e in range(2)]
    for mo in range(NMO):
        m = NSZ[mo]
        pst = ps.tile([128, 2 * L], F32, tag="acc")
        for ki in range(KD):
            nc.tensor.matmul(pst[:m, :], w.qWT[:, ki, NOFF[mo]:NOFF[mo] + m],
                             st.x2T[:, ki, :], start=(ki == 0), stop=(ki == KD - 1))
        zst = act.tile([128, 2 * L], BF16, tag="scr1", bufs=3,
                       name=f"zst_{p}_{mo}")
        if mo % 2 == 0:
            nc.scalar.activation(zst[:m, :], pst[:m, :], AF.Identity,
                                 bias=w.zbcol[:m, mo:mo + 1])
        else:
            nc.vector.tensor_scalar(out=zst[:m, :], in0=pst[:m, :],
                                    scalar1=w.zbcol[:m, mo:mo + 1],
                                    scalar2=None, op0=OP.add)
        for e in range(2):
            nc.sync.dma_start(zf[e][NOFF[mo]:NOFF[mo] + m, :],
                              zst[:m, e * L:(e + 1) * L])
    # gathers: z rows then bexp rows (gpsimd queue order matters)
    for e in range(2):
        for mo in range(NMO):
            m = NSZ[mo]
            nc.gpsimd.indirect_dma_start(
                out=st.z[e][:m, mo, :], out_offset=None, in_=zf[e][:, :],
                in_offset=bass.IndirectOffsetOnAxis(
                    ap=st.idx[e][:m, mo:mo + 1], axis=0))

    return


def _sel_part(nc, p, st, act, small, ps, w):
    """az/bz + row sums; emb/sel matmuls + drains."""
    # ---- az/bz + row sums ----
    st.az = []
    st.bz = []
    st.azT = []
    st.bzT = []
    st.rfw = []
    for e in range(2):
        az = act.tile([128, NMO, L], BF16, tag=f"az{e}", bufs=1)
        bz = act.tile([128, NMO, L], BF16, tag=f"bz{e}", bufs=1)
        sum_a = small.tile([128, NMO], F32, tag="sum_a")
        sum_b = small.tile([128, NMO], F32, tag="sum_b")
        nc.vector.memset(sum_a[:], 1.0)
        nc.vector.memset(sum_b[:], 1.0)
        for mo in range(NMO):
            m = NSZ[mo]
            nc.vector.scalar_tensor_tensor(
                out=az[:m, mo, :], in0=st.z[e][:m, mo, :], scalar=0.0,
                in1=st.mask[e][:m, mo, :], op0=OP.max, op1=OP.mult,
                accum_out=sum_a[:m, mo:mo + 1])
            nc.vector.scalar_tensor_tensor(
                out=bz[:m, mo, :], in0=st.z[e][:m, mo, :], scalar=0.0,
                in1=st.mask[e][:m, mo, :], op0=OP.min, op1=OP.mult,
                accum_out=sum_b[:m, mo:mo + 1])
        rfw_a = small.tile([128, NMO], F32, tag="rfw_a")
        rfw_b = small.tile([128, NMO], F32, tag="rfw_b")
        tmp_a = small.tile([128, NMO], F32, tag="tmp_a")
        tmp_b = small.tile([128, NMO], F32, tag="tmp_b")
        nc.vector.tensor_scalar_add(tmp_a[:], sum_a[:], EPS)
        nc.vector.reciprocal(rfw_a[:], tmp_a[:])
        nc.vector.tensor_scalar_add(tmp_b[:], sum_b[:], -EPS)
        nc.vector.reciprocal(rfw_b[:], tmp_b[:])
        st.rfw.append((rfw_a, rfw_b))

        st.az.append(az)
        st.bz.append(bz)

    # ---- emb/sel matmuls: 5 streams share each stationary x2T slice ----
    st.emb = []
    st.sel = []
    for e in range(2):
        emb_a = act.tile([128, LT, D], BF16, tag=f"emb_a{e}", bufs=1)
        emb_b = act.tile([128, LT, D], BF16, tag=f"emb_b{e}", bufs=1)
        sel = act.tile([128, LT, D], BF16, tag=f"sel{e}", bufs=1)
        for lt in range(LT):
            grp = [ps.tile([128, D], F32, tag="acc", name=f"emb_{p}_{e}_{lt}_{g}")
                   for g in range(5)]
            for ki in range(KD):
                lhs = st.x2T[:, ki, e * L + lt * 128:e * L + lt * 128 + 128]
                for g, wi in enumerate((W_A, W_GA, W_B, W_GB, W_S)):
                    nc.tensor.matmul(grp[g][:], lhs, w.w5[:, wi, ki, :],
                                     start=(ki == 0), stop=False)
            for g, bi in ((1, B_GA), (3, B_GB), (4, B_S), (0, B_A), (2, B_B)):
                nc.tensor.matmul(grp[g][:], w.ones_row[:], w.bias6[:, bi, :],
                                 start=False, stop=True)
            # drains: sigmoids first (ACT), then the DVE mults
            sigs = []
            for g_gate in (1, 3):
                sig = act.tile([128, D], BF16, tag="scr1", bufs=3,
                               name=f"sig_{p}_{e}_{lt}_{g_gate}")
                nc.scalar.activation(sig[:], grp[g_gate][:], AF.Sigmoid)
                sigs.append(sig)
            nc.scalar.activation(sel[:, lt, :], grp[4][:], AF.Sigmoid)
            for side, g_emb in enumerate((0, 2)):
                dst = emb_a if side == 0 else emb_b
                nc.vector.tensor_tensor(out=dst[:, lt, :], in0=grp[g_emb][:],
                                        in1=sigs[side][:], op=OP.mult)
        st.emb.append((emb_a, emb_b))
        st.sel.append(sel)
    return st


def _mid(nc, p, st, act, small, ps, pstr, w, x_d):
    """cfw + bw matmuls for both sides/elements, combine into y2."""
    st.y2 = []
    st.azT = []
    st.bzT = []
    st.rbw = []
    st.xr = []
    for e in range(2):
        xr = act.tile([128, LT, D], BF16, tag=f"xr{e}", bufs=1,
                      name=f"xr_{p}_{e}")
        nc.sync.dma_start(xr[:], w.xbf_d[st.bs[e]].rearrange("(lt p) d -> p lt d",
                                                             p=128))
        st.xr.append(xr)
    for e in range(2):
        azT = act.tile([128, LT, NMO, 128], BF16, tag=f"azT{e}", bufs=1,
                       name=f"azT_{p}_{e}")
        bzT = act.tile([128, LT, NMO, 128], BF16, tag=f"bzT{e}", bufs=1,
                       name=f"bzT_{p}_{e}")
        for side, (zz, zzT) in enumerate(((st.az[e], azT), (st.bz[e], bzT))):
            cpeng = side
            for lt in range(LT):
                for half in range(2):
                    ptr = pstr.tile([128, 4, 128], BF16, tag="tr",
                                    name=f"trz_{p}_{e}_{side}_{lt}_{half}")
                    for mi in range(4):
                        mo = half * 4 + mi
                        m = NSZ[mo]
                        nc.tensor.transpose(
                            ptr[:, mi, :m],
                            zz[:m, mo, lt * 128:(lt + 1) * 128],
                            w.ident[:m, :m])
                    if cpeng == 0:
                        nc.vector.tensor_copy(
                            zzT[:, lt, half * 4:half * 4 + 4, :], ptr[:, :, :])
                    else:
                        nc.scalar.copy(
                            zzT[:, lt, half * 4:half * 4 + 4, :], ptr[:, :, :])
                    if half == 1:
                        nc.vector.memset(zzT[:, lt, 7, 96:128], 0.0)
        st.azT.append(azT)
        st.bzT.append(bzT)
        st.rbw.append(None)  # filled between cfw and bw emission

    for e in range(2):
        # cfw[n, d] = rfw[n] * sum_l zzT[l, n]^T emb[l, d] + bexp
        # (sides interleaved per mo so the rotating bexp tile dies early)
        cfws = [act.tile([128, NMO, D], BF16, tag=f"cfw{s}", bufs=1,
                         name=f"cfw_{p}_{e}_{s}") for s in range(2)]
        for mo in range(NMO):
            m = NSZ[mo]
            bx = act.tile([128, D], BF16, tag="bexp", bufs=4,
                          name=f"bexp_{p}_{e}_{mo}")
            nc.gpsimd.indirect_dma_start(
                out=bx[:m, :], out_offset=None, in_=w.btab_d[:, :],
                in_offset=bass.IndirectOffsetOnAxis(
                    ap=st.idx[e][:m, mo:mo + 1], axis=0))
            psts = []
            for side in range(2):
                zzT = st.azT[e] if side == 0 else st.bzT[e]
                emb = st.emb[e][side]
                pst = ps.tile([128, D], F32, tag="acc",
                              name=f"psc_{p}_{e}_{mo}_{side}")
                for lt in range(LT):
                    nc.tensor.matmul(pst[:m, :],
                                     zzT[:, lt, mo, :m],
                                     emb[:, lt, :], start=(lt == 0),
                                     stop=(lt == LT - 1))
                psts.append(pst)
            for side in range(2):
                nc.vector.scalar_tensor_tensor(
                    out=cfws[side][:m, mo, :], in0=psts[side][:m, :],
                    scalar=st.rfw[e][side][:m, mo:mo + 1],
                    in1=bx[:m, :],
                    op0=OP.mult, op1=OP.add)

        azT, bzT = st.azT[e], st.bzT[e]
        den_a = small.tile([128, LT], F32, tag="den_a")
        den_b = small.tile([128, LT], F32, tag="den_b")
        nc.vector.tensor_reduce(den_a[:], azT[:, :, :, :], axis=AX.XY, op=OP.add)
        nc.vector.tensor_reduce(den_b[:], bzT[:, :, :, :], axis=AX.XY, op=OP.add)
        rbw_a = small.tile([128, LT], F32, tag="rbw_a")
        rbw_b = small.tile([128, LT], F32, tag="rbw_b")
        t2a = small.tile([128, LT], F32, tag="t2a")
        t2b = small.tile([128, LT], F32, tag="t2b")
        nc.vector.tensor_scalar_add(t2a[:], den_a[:], EPS)
        nc.vector.reciprocal(rbw_a[:], t2a[:])
        nc.vector.tensor_scalar_add(t2b[:], den_b[:], -EPS)
        nc.vector.reciprocal(rbw_b[:], t2b[:])
        st.rbw[e] = (rbw_a, rbw_b)

        outs = []
        for side in range(2):
            zz = st.az[e] if side == 0 else st.bz[e]
            rbw = st.rbw[e][side]
            cfw = cfws[side]
            # bw: out[l, d] = rbw[l] * sum_n zz[n, l] cfw[n, d]
            out_raw = act.tile([128, LT, D], BF16, tag=f"out_{side}", bufs=1,
                               name=f"outr_{p}_{e}_{side}")
            for lt in range(LT):
                pst = ps.tile([128, D], F32, tag="acc")
                for mo in range(NMO):
                    m = NSZ[mo]
                    nc.tensor.matmul(pst[:],
                                     zz[:m, mo, lt * 128:(lt + 1) * 128],
                                     cfw[:m, mo, :], start=(mo == 0),
                                     stop=(mo == NMO - 1))
                if lt == 0:
                    nc.scalar.activation(out_raw[:, lt, :], pst[:], AF.Identity,
                                         scale=rbw[:, lt:lt + 1])
                else:
                    nc.vector.tensor_scalar(out=out_raw[:, lt, :], in0=pst[:],
                                            scalar1=rbw[:, lt:lt + 1],
                                            scalar2=None, op0=OP.mult)
            outs.append(out_raw)

        # combine: y2 = x + out_b + sel * (out_a - out_b)
        out_a, out_b = outs
        y2 = act.tile([128, LT, D], BF16, tag=f"y2_{e}", bufs=1,
                      name=f"y2_{p}_{e}")
        for lt in range(LT):
            dt_ = act.tile([128, D], BF16, tag="scr1", bufs=3)
            nc.vector.tensor_tensor(out=dt_[:], in0=out_a[:, lt, :],
                                    in1=out_b[:, lt, :], op=OP.subtract)
            nc.vector.tensor_tensor(out=dt_[:], in0=dt_[:],
                                    in1=st.sel[e][:, lt, :], op=OP.mult)
            nc.gpsimd.tensor_tensor(out=y2[:, lt, :], in0=st.xr[e][:, lt, :],
                                    in1=out_b[:, lt, :], op=OP.add)
            nc.vector.tensor_tensor(out=y2[:, lt, :], in0=y2[:, lt, :],
                                    in1=dt_[:], op=OP.add)
        st.y2.append(y2)


def _ff(nc, st, act, small, ps, pstr, w, out_d):
    """LN2 + feed-forward + residual for both elements of the pair."""
    p = st.p
    # LN2 -> x3 -> x3T (pair-interleaved)
    x3T = act.tile([128, KD, 2 * L], BF16, tag="x3T", bufs=1)
    for e in range(2):
        x3 = act.tile([128, LT, D], BF16, tag=f"x3_{e}", bufs=1)
        _layer_norm(nc, small, x3, st.y2[e], w.eps, f"ln2_{p}_{e}")
        for lt in range(LT):
            ptr = pstr.tile([128, KD, 128], BF16, tag="tr",
                            name=f"trf_{p}_{e}_{lt}")
            for ko in range(KD):
                nc.tensor.transpose(ptr[:, ko, :],
                                    x3[:, lt, ko * 128:(ko + 1) * 128],
                                    w.ident[:])
            off = e * L + lt * 128
            if lt == 0:
                nc.vector.tensor_copy(x3T[:, :, off:off + 128], ptr[:, :, :])
            else:
                nc.scalar.copy(x3T[:, :, off:off + 128], ptr[:, :, :])

    # hT (pair): relu(x3 @ Wf1 + bf1)^T : [dff-part, KF, 2L]
    hT = act.tile([128, KF, 2 * L], BF16, tag="hT", bufs=1)
    for mo in range(KF):
        pst = ps.tile([128, 2 * L], F32, tag="acc")
        for ki in range(KD):
            nc.tensor.matmul(pst[:], w.wf1[:, ki, mo * 128:(mo + 1) * 128],
                             x3T[:, ki, :], start=(ki == 0), stop=(ki == KD - 1))
        if mo % 2 == 0:
            nc.scalar.activation(hT[:, mo, :], pst[:], AF.Relu,
                                 bias=w.bf1col[:, mo:mo + 1])
        else:
            nc.vector.tensor_scalar(out=hT[:, mo, :], in0=pst[:],
                                    scalar1=w.bf1col[:, mo:mo + 1],
                                    scalar2=0.0, op0=OP.add, op1=OP.max)

    # ffout per element: out = y2 + hT^T @ Wf2 + bf2
    for e in range(2):
        osb = act.tile([128, LT, D], BF16, tag=f"osb{e}", bufs=1)
        for lt in range(LT):
            pst = ps.tile([128, D], F32, tag="acc")
            off = e * L + lt * 128
            for mo in range(KF):
                nc.tensor.matmul(pst[:], hT[:, mo, off:off + 128],
                                 w.wf2[:, mo, :], start=(mo == 0),
                                 stop=False)
            nc.tensor.matmul(pst[:], w.ones_row[:], w.bias6[:, B_F2, :],
                             start=False, stop=True)
            nc.vector.scalar_tensor_tensor(
                out=osb[:, lt, :], in0=pst[:], scalar=1.0,
                in1=st.y2[e][:, lt, :], op0=OP.mult, op1=OP.add)
        nc.sync.dma_start(out_d[st.bs[e]].rearrange("(lt p) d -> p lt d", p=128),
                          osb[:])


# ---------------------------------------------------------------------------
# host-side weight preprocessing + SPMD launch
# ---------------------------------------------------------------------------

def _prep_host(inputs):
    f = lambda k: np.ascontiguousarray(np.asarray(inputs[k], dtype=np.float32))
    g1, b1 = f("ln1_g"), f("ln1_b")
    g2, b2 = f("ln2_g"), f("ln2_b")
    Wk, bk = f("Wk"), f("bk")
    Wa, ba = f("Wa"), f("ba")
    Wa1, ba1 = f("Wa1"), f("ba1")
    Wb, bb = f("Wb"), f("bb")
    Wb1, bb1 = f("Wb1"), f("bb1")
    Ws, bsel = f("Ws"), f("bsel")
    Wf1, bf1 = f("Wf1"), f("bf1")
    Wf2, bf2 = f("Wf2"), f("bf2")
    q_tab, b_tab = f("q_tab"), f("b_tab")

    qs = q_tab / np.sqrt(np.float32(D))
    qW = qs @ (g1[:, None] * Wk).T                 # [N, D]
    zb = qs @ (b1 @ Wk + bk)                       # [N]
    qWT = np.ascontiguousarray(qW.T)               # [D, N]

    Waa1 = Wa @ Wa1
    Wbb1 = Wb @ Wb1
    w5 = np.stack([
        g1[:, None] * Wa,
        g1[:, None] * Waa1,
        g1[:, None] * Wb,
        g1[:, None] * Wbb1,
        g1[:, None] * Ws,
    ]).astype(np.float32)
    bias6 = np.stack([
        b1 @ Wa + ba,
        b1 @ Waa1 + ba @ Wa1 + ba1,
        b1 @ Wb + bb,
        b1 @ Wbb1 + bb @ Wb1 + bb1,
        b1 @ Ws + bsel,
        bf2,
    ]).astype(np.float32)                          # [6, D]
    bias6_bcast = np.broadcast_to(bias6[None, :, :], (128, 6, D))

    zb_col = np.zeros((128, NMO), np.float32)
    zb_pad = np.zeros((NPAD,), np.float32)
    zb_pad[:N] = zb
    for mo in range(NMO):
        zb_col[:, mo] = zb_pad[mo * 128:(mo + 1) * 128]

    wf1 = g2[:, None] * Wf1
    bf1p = (b2 @ Wf1 + bf1).astype(np.float32)     # [DFF]
    bf1_col = np.zeros((128, KF), np.float32)
    for mo in range(KF):
        bf1_col[:, mo] = bf1p[mo * 128:(mo + 1) * 128]

    bf = lambda a: np.ascontiguousarray(np.asarray(a, dtype=np_bf16))
    return dict(
        ident=np.ascontiguousarray(np.eye(128, dtype=np_bf16)),
        qWT=bf(qWT), w5=bf(w5), b_tab=bf(b_tab),
        bias6=bf(bias6_bcast), zbcol=np.ascontiguousarray(zb_col),
        bf1col=np.ascontiguousarray(bf1_col),
        wf1=bf(wf1), wf2=bf(Wf2),
    )


_NC_CACHE = {}


def _get_program(n_elems=BPC):
    if n_elems not in _NC_CACHE:
        _NC_CACHE[n_elems] = build_program(n_elems)
    return _NC_CACHE[n_elems]


def make_in_maps(inputs):
    x = np.ascontiguousarray(np.asarray(inputs["x"], dtype=np.float32))
    nidx = np.asarray(inputs["n_indexes"]).astype(np.int32)
    nidx_pad = np.zeros((BS, NPAD), np.int32)
    nidx_pad[:, :N] = nidx
    mask = (np.asarray(inputs["mask"]) != 0)
    mask_pad = np.zeros((BS, NPAD, L), np_bf16)
    mask_pad[:, :N, :] = mask.astype(np_bf16)
    shared = _prep_host(inputs)
    in_maps = []
    for c in range(NCORES):
        sl = slice(c * BPC, (c + 1) * BPC)
        in_maps.append({
            "x": np.ascontiguousarray(x[sl]),
            "xbf": np.ascontiguousarray(x[sl].astype(np_bf16)),
            "nidx": np.ascontiguousarray(nidx_pad[sl]),
            "mask": np.ascontiguousarray(mask_pad[sl]),
            **shared,
        })
    return in_maps


def kernel(**inputs):
    from concourse.bass_utils import run_bass_kernel_spmd

    nc = _get_program(BPC)
    in_maps = make_in_maps(inputs)
    res = run_bass_kernel_spmd(nc, in_maps, core_ids=list(range(NCORES)))
    out = np.concatenate([res.results[c]["out"] for c in range(NCORES)], axis=0)
    return out.astype(np.float32)
